# revision 48
# baseline (speedup 1.0000x reference)
"""Trainium2 Bass kernel for nn_ContrastiveLoss (ragged_sequence), v2.

Math (see reference): a cross-attention t2i score matrix scores[i, c] over
B=64 images x B=64 captions, then a max-violation margin loss.

Sharding: caption slots are sharded 8-per-core across 8 NeuronCores with a
four-width ragged slot layout (slot widths chosen per call from s_l and
compiled per layout); images are replicated.  Each core computes its
[64, 8] slot block of the score matrix; the host un-permutes slots and
runs the tiny margin reduction.

v2 design (~68us baseline -> target ~40us):
  * all A matmuls fp8e4 x fp8e4 (im, s, G/8 shipped fp8; measured loss
    error 1.7e-5); DMA traffic drops 8.5MB -> ~3.5MB per core.
  * four-width slot packing: NCW ~300 -> ~244 columns per core.
  * ea|eh concatenated per pack -> ONE accumulation matmul per pack
    (nz|wz side by side in a single PSUM bank); 20 matmuls per pair.
  * ones stationaries for the r-reduction are shifted 128-col windows of
    one tiny [108, 191] constant (nothing shipped per pack).
  * no raw-A Copy: ea reads A straight from PSUM (a_ps lives ~4 steps;
    PSUM budget 4+2+1 banks of 8).
  * engine split per pair: PE 20 matmuls; ACT Prelu+Exp; GPSIMD
    sq/newton-rsqrt/an; DVE word-norm reduces + ea + eh.
  * ~26 warmup matmuls on zeros during the input DMA keep HAM's clock
    gate busy so the first real burst runs at 2.4 GHz.
  * phase 2 (sim/LSE epilogue) is column-halved with the rsqrt newton on
    GPSIMD and the rest on DVE so semaphore latency overlaps.
  * Z-cancellation: sim = nz/(cn*sqrt(wz)) -- the softmax denominator
    cancels, so no Z accumulator, no reciprocals.
"""

import sys

if "/opt/trn_rl_repo" not in sys.path:
    sys.path.insert(0, "/opt/trn_rl_repo")

import numpy as np

B, R, W, D = 64, 36, 50, 1024
NCORES = 8
CPC = B // NCORES          # caption slots per core = 8
PACK = 3                   # images per pack
NPACK = 22                 # ceil(64 / 3) -> 66 rows incl 2 pad images
BP = NPACK * PACK          # 66
PPART = PACK * R           # 108 partitions per pack
KCH = D // 128             # 8 contraction chunks
SW = 128
IMC = KCH * SW             # 1024 im columns per pack (108 real + 20 pad)
PKC = IMC + SW             # 1152 pack columns (im | G)
WOFF = 63                  # onesbase window base column
GSCALE = 8.0               # G is shipped as G/8 (fp8e4 max ~240)
WARM_MM = 30               # PE warmup matmuls during input DMA

MARGIN = 0.2
LAM_SM = 9.0
LAM_LSE = 6.0
CLAMP_INT = 0x1E3CE508     # bits of f32 1e-20 (newton-seed zero guard)

_PROGRAM_CACHE: dict = {}

_RSQ_NAME = "ANT_RSQRT_NSTEP"


def _rsq_op():
    """Fused rsqrt Newton step as a custom DVE op:
    out = (sq(in1)*s0*in0 + s1)*in1  -- with in1 = magic-seed y0, in0 = x,
    (s0, s1) = (-4.5, 13.5) this is one Newton iteration of 9/sqrt(x).
    Registered into dve_ops.OPS at first use (documented extension point)."""
    import numpy as np
    import concourse.dve_ops as dve_ops
    for op in dve_ops.OPS:
        if op.name == _RSQ_NAME:
            return op
    from concourse.dve_spec import Spec, Src0, Src1, C0, C1, sq, lower
    from concourse.dve_spec import _has_src1
    from concourse.dve_uop import DveOpSpec

    # (sq(y0)*x)*c0 + c1)*y0 -- sq*x FIRST so x=0 (pad columns) zeroes the
    # huge seed before the c0 scale can overflow to inf
    spec = Spec(
        body=(sq(Src1) * Src0 * C0 + C1) * Src1,
        reference=lambda in0, in1, c0, c1, c2: (
            (in1.astype(np.float32) ** 2 * in0 * c0 + c1) * in1
        ),
    )
    row = dve_ops._CUSTOM_DVE_ROW_BASE + len(dve_ops.OPS)
    shas = {}
    for ver in ("v3", "v4"):
        s = DveOpSpec(name=_RSQ_NAME, opcode=row, uops=lower(spec, ver=ver),
                      rd1_en=_has_src1(spec))
        shas[ver] = s.sha(ver)
    op = dve_ops.DveOp(_RSQ_NAME, spec, subdim=False, uops_sha=shas)
    dve_ops.OPS.append(op)
    dve_ops._SUB_OPCODE_FOR_NAME[_RSQ_NAME] = row
    dve_ops.CUSTOM_DVE_SPECS[_RSQ_NAME] = spec
    return op


def choose_layout(s_l: np.ndarray):
    """Four-width caption slot packing.  Captions sorted by length are
    dealt round-robin: core c, slot k holds caption order[k*8+c].  Slot
    rank k needs width >= len(order[k*8+7]); ranks are grouped into <= 4
    contiguous groups sharing one (even) width, chosen to minimize NCW.
    Returns (widths, caps) with widths[k] = slot k's width."""
    s_l = np.asarray(s_l).astype(np.int64)
    order = np.argsort(s_l, kind="stable")
    caps = np.empty((NCORES, CPC), np.int64)
    for k in range(CPC):
        for c in range(NCORES):
            caps[c, k] = order[k * NCORES + c]
    need = [int(s_l[order[k * NCORES + NCORES - 1]]) for k in range(CPC)]

    best = None
    import itertools
    for nsplit in range(4):
        for cuts in itertools.combinations(range(1, CPC), nsplit):
            bounds = (0,) + cuts + (CPC,)
            widths = []
            for a, b in zip(bounds[:-1], bounds[1:]):
                w = max(need[a:b])
                widths += [w] * (b - a)
            ncw = sum(widths)
            if best is None or (ncw, nsplit) < best[0]:
                best = ((ncw, nsplit), tuple(widths))
    return best[1], caps


def slot_groups(widths):
    """Contiguous equal-width slot groups: list of (k0, nslots, w, col0)."""
    groups = []
    col = 0
    k = 0
    while k < CPC:
        k2 = k
        while k2 < CPC and widths[k2] == widths[k]:
            k2 += 1
        groups.append((k, k2 - k, widths[k], col))
        col += (k2 - k) * widths[k]
        k = k2
    return groups


def build_program(widths, debug: bool = False):
    import concourse.bacc as bacc
    import concourse.mybir as mybir
    import concourse.tile as tile

    f32 = mybir.dt.float32
    bf16 = mybir.dt.bfloat16
    fp8 = mybir.dt.float8e4
    i32 = mybir.dt.int32
    AF = mybir.ActivationFunctionType
    ALU = mybir.AluOpType
    AX = mybir.AxisListType

    NCW = sum(widths)
    groups = slot_groups(widths)
    U = 2 if NCW <= 256 else 1          # packs per step
    PBS = 256 if NCW <= 256 else 512    # per-pack PSUM stride
    NST = NPACK // U                    # pipeline steps (11 or 22)
    ACC1 = (2 * NCW <= 512)             # nz|wz share one PSUM bank

    # newton runs per step (singles): rn9(j) is ready at the end of step j,
    # so an(j) can run at step j+1 with a full step of slack
    PG = [[t] for t in range(NST)]
    group_of = {j: j for j in range(NST)}
    NMAX = U * CPC

    rsq = _rsq_op()

    nc = bacc.Bacc("TRN2", target_bir_lowering=False, debug=debug)

    pk_d = nc.dram_tensor("pk8", [NST, 128, U * PKC], fp8, kind="ExternalInput")
    s_d = nc.dram_tensor("s8", [128, KCH * NCW], fp8, kind="ExternalInput")
    ones_d = nc.dram_tensor("onesb", [128, WOFF + 128], bf16, kind="ExternalInput")
    nlc_d = nc.dram_tensor("nlcv", [BP, NCW], f32, kind="ExternalInput")
    pc_d = nc.dram_tensor("padcv", [BP, CPC], f32, kind="ExternalInput")
    out_d = nc.dram_tensor("scores8", [B, CPC], f32, kind="ExternalOutput")

    with tile.TileContext(nc) as tc:
        with (
            tc.tile_pool(name="const", bufs=1) as cpool,
            tc.tile_pool(name="pk", bufs=6) as pkpool,
            tc.tile_pool(name="ala", bufs=4) as alpool,
            tc.tile_pool(name="sqp", bufs=3) as sqpool,
            tc.tile_pool(name="anp", bufs=3) as anpool,
            tc.tile_pool(name="ep", bufs=4) as epool,
            tc.tile_pool(name="eaeh", bufs=3) as eapool,
            tc.tile_pool(name="nwt", bufs=2) as nwt,
            tc.tile_pool(name="ph2", bufs=2) as ph2,
            tc.tile_pool(name="psA", bufs=4, space="PSUM") as psA,
            tc.tile_pool(name="psH", bufs=2, space="PSUM") as psH,
            tc.tile_pool(name="psacc", bufs=1, space="PSUM") as psacc,
        ):
            s_sb = cpool.tile([128, KCH * NCW], fp8)
            ones_sb = cpool.tile([128, WOFF + 128], bf16)
            nlc_sb = cpool.tile([BP, NCW], f32)
            pc_sb = cpool.tile([BP, CPC], f32)

            magic = cpool.tile([PPART, 1], i32)
            nc.vector.memset(magic[:], 0x5F3759DF)
            wconst = cpool.tile([128, 128], bf16)
            nc.vector.memset(wconst[:], 0.0)

            s2_all = cpool.tile([PPART, NPACK * CPC], f32)
            rn9_all = cpool.tile([PPART, NPACK * CPC], f32)

            # persistent accumulators: [nz | wz] in one PSUM bank
            if ACC1:
                acc = psacc.tile([128, 2 * NCW], f32)
                nz_v = acc[:, 0:NCW]
                wz_v = acc[:, NCW:2 * NCW]
            else:
                acc_n = psacc.tile([128, NCW], f32)
                acc_w = psacc.tile([128, NCW], f32)
                nz_v, wz_v = acc_n[:], acc_w[:]
            wfill = psacc.tile([128, 128], f32, tag="wfill")

            # warmup matmuls on zeros: keep the PE's HAM activity window
            # busy while the first input DMAs land
            for _ in range(WARM_MM):
                nc.tensor.matmul(wfill[:], wconst[:], wconst[:],
                                 start=True, stop=True)

            def pe_fill(n):
                """Dependency-free matmuls into a scratch bank: absorb the
                PE idle gap while the burst waits on its PSUM tile so the
                HAM clock gate never sees an idle window (cold = half clock)."""
                for _ in range(n):
                    nc.tensor.matmul(wfill[:], wconst[:], wconst[:],
                                     start=True, stop=True)

            al_t: dict = {}
            pk_t: dict = {}
            e_t: dict = {}
            a_ps_t: dict = {}

            def pview(t_, n=NCW):
                return t_[0:PPART].rearrange(
                    "p (u x) -> p u x", u=U)[:, :, 0:n]

            def dma_pk(j):
                pk_sb = pkpool.tile([128, U * PKC], fp8, tag="pk")
                nc.sync.dma_start(pk_sb[:], pk_d[j])
                pk_t[j] = pk_sb

            def sweep1_mm(j):
                if j == 0:
                    nc.sync.dma_start(s_sb[:, :2 * NCW], s_d[:, :2 * NCW])
                    dma_pk(0)
                    dma_pk(1)
                    nc.sync.dma_start(ones_sb[:], ones_d[:])
                elif j + 1 < NST:
                    dma_pk(j + 1)
                if j == 1:
                    nc.sync.dma_start(nlc_sb[:], nlc_d[:])
                    nc.sync.dma_start(pc_sb[:], pc_d[:])
                pk_sb = pk_t[j]
                a_ps = psA.tile([128, U * PBS], f32)
                if j == 0:
                    for k in range(KCH):
                        if k == 2:
                            nc.sync.dma_start(s_sb[:, 2 * NCW:5 * NCW],
                                              s_d[:, 2 * NCW:5 * NCW])
                        if k == 5:
                            nc.sync.dma_start(s_sb[:, 5 * NCW:],
                                              s_d[:, 5 * NCW:])
                        for u in range(U):
                            nc.tensor.matmul(
                                a_ps[:, u * PBS:u * PBS + NCW],
                                pk_sb[:, u * PKC + k * SW:u * PKC + (k + 1) * SW],
                                s_sb[:, k * NCW:(k + 1) * NCW],
                                start=(k == 0), stop=(k == KCH - 1),
                            )
                else:
                    for u in range(U):
                        for k in range(KCH):
                            nc.tensor.matmul(
                                a_ps[:, u * PBS:u * PBS + NCW],
                                pk_sb[:, u * PKC + k * SW:u * PKC + (k + 1) * SW],
                                s_sb[:, k * NCW:(k + 1) * NCW],
                                start=(k == 0), stop=(k == KCH - 1),
                            )
                a_ps_t[j] = a_ps

            def sweep1_post(j):
                a_ps = a_ps_t[j]
                # al = leaky_relu(A, 0.1) (ACT, PSUM -> SBUF bf16)
                al = alpool.tile([PPART, U * NCW], bf16, tag="al")
                nc.scalar.activation(
                    al[:].rearrange("p (u x) -> p u x", u=U),
                    pview(a_ps), AF.Prelu, alpha=0.1,
                )
                al_t[j] = al
                # sq = al^2 (ACT Square), s2 = per-slot word sums (DVE)
                # sq split ACT/DVE at the u-boundary to balance the two
                # engines (ACT gained vv, DVE lost eh)
                sq = sqpool.tile([PPART, U * NCW], bf16, tag="sq")
                nc.scalar.activation(sq[:, 0:NCW], al[:, 0:NCW], AF.Square)
                if U > 1:
                    nc.vector.tensor_mul(sq[:, NCW:], al[:, NCW:],
                                         al[:, NCW:])
                sqr = sq[:].rearrange("p (u x) -> p u x", u=U)
                s2r = s2_all[:, j * U * CPC:(j + 1) * U * CPC].rearrange(
                    "p (u c) -> p u c", u=U)
                for (k0, ns, w, c0) in groups:
                    nc.vector.tensor_reduce(
                        s2r[:, :, k0:k0 + ns],
                        sqr[:, :, c0:c0 + ns * w].rearrange(
                            "p u (c w) -> p u c w", c=ns),
                        AX.X, ALU.add,
                    )

            def newton(g):
                """rn9 = 9/sqrt(s2): fused int-clamp+shift (one
                tensor_scalar: int-max == float-max for positive floats),
                magic seed subtract, then ONE fused custom-DVE Newton step.
                The clamp keeps all-zero pad-image columns finite."""
                j = PG[g][0]
                lo, hi = j * U * CPC, (j + 1) * U * CPC
                n = hi - lo

                def tl(tag, dt=f32):
                    t_ = nwt.tile([PPART, NMAX], dt, tag=tag, name=f"nwt_{tag}")
                    return t_[:, :n]

                t1 = tl("t1", i32)
                nc.vector.tensor_scalar(
                    t1, s2_all[:, lo:hi].bitcast(i32), 1, 1,
                    op0=ALU.bitwise_or, op1=ALU.logical_shift_right
                )
                y0 = tl("y0")
                nc.gpsimd.tensor_tensor(
                    y0.bitcast(i32),
                    magic[:].broadcast_to([PPART, n]),
                    t1,
                    op=ALU.subtract,
                )
                nc.vector._custom_dve(
                    rsq, out=rn9_all[:, lo:hi], in0=s2_all[:, lo:hi],
                    in1=y0, s0=-4.5, s1=13.5,
                )

            an_t: dict = {}

            def sweep2_an(j):
                # an = al * rn9 broadcast over words (GPSIMD, per width-group).
                # For the tail pairs (no bursts left) the width-groups split
                # across GPSIMD and DVE: DVE runs disjoint-region writes
                # back-to-back, halving the an chain that gates exp -> H.
                al = al_t.pop(j)
                an = anpool.tile([PPART, U * NCW], f32, tag="an")
                anr = an[:].rearrange("p (u x) -> p u x", u=U)
                alr = al[:].rearrange("p (u x) -> p u x", u=U)
                rn = rn9_all[:, j * U * CPC:(j + 1) * U * CPC].rearrange(
                    "p (u c) -> p u c", u=U)
                for gi, (k0, ns, w, c0) in enumerate(groups):
                    eng = nc.gpsimd
                    eng.tensor_mul(
                        anr[:, :, c0:c0 + ns * w].rearrange(
                            "p u (c w) -> p u c w", c=ns),
                        alr[:, :, c0:c0 + ns * w].rearrange(
                            "p u (c w) -> p u c w", c=ns),
                        rn[:, :, k0:k0 + ns].broadcast_to([PPART, U, ns, w]),
                    )
                an_t[j] = an

            def sweep2_exp(j):
                an = an_t.pop(j)
                e = epool.tile([PPART, U * NCW], bf16, tag="e")
                nc.scalar.activation(e[:], an[:], AF.Exp)
                e_t[j] = e

            rest_st: dict = {}

            def sweep2_rest_a(j):
                """H matmuls + ea (DVE) -- emitted BEFORE sweep1_post(t) so
                the DVE queue runs ea/reduces/newton while the PE works
                toward H; eh (which needs H) is emitted after them."""
                pk_sb, e = pk_t.pop(j), e_t[j]
                a_ps = a_ps_t.pop(j)

                # H = (G/8) @ E per pack (fp8 stationary x bf16 moving)
                h_ps = psH.tile([128, U * PBS], f32, tag="h_ps")
                for u in range(U):
                    nc.tensor.matmul(
                        h_ps[:, u * PBS:u * PBS + NCW],
                        pk_sb[:PPART, u * PKC + IMC:u * PKC + IMC + SW],
                        e[:, u * NCW:(u + 1) * NCW],
                        start=True, stop=True,
                    )
                # ea | eh concatenated per pack: [108, U, 2, NCW]
                ee4 = eapool.tile([PPART, U * 2 * NCW], bf16, tag="ee4")
                er4 = ee4[:].rearrange("p (u k x) -> p u k x", u=U, k=2)
                nc.vector.tensor_mul(
                    er4[:, :, 0, :],
                    e[:].rearrange("p (u x) -> p u x", u=U),
                    pview(a_ps),
                )
                rest_st[j] = (e, h_ps, ee4)

            def sweep2_rest_b(j):
                e, h_ps, ee4 = rest_st.pop(j)
                e_t.pop(j)
                er4 = ee4[:].rearrange("p (u k x) -> p u k x", u=U, k=2)
                # vv = V^2 on ACT (was eh = e*H on DVE -- the Cholesky form
                # moves this pass off the DVE pacer)
                nc.scalar.activation(er4[:, :, 1, :], pview(h_ps), AF.Square)
                # one accumulation matmul per pack: [nz | wz] += ones^T [ea|eh]
                for u in range(U):
                    p = U * j + u
                    wcol = WOFF - PACK * p
                    if ACC1:
                        nc.tensor.matmul(
                            acc[:],
                            ones_sb[0:PPART, wcol:wcol + 128],
                            ee4[:, u * 2 * NCW:(u + 1) * 2 * NCW],
                            start=(p == 0), stop=(p == NPACK - 1),
                        )
                    else:
                        nc.tensor.matmul(
                            acc_n[:],
                            ones_sb[0:PPART, wcol:wcol + 128],
                            ee4[:, u * 2 * NCW:u * 2 * NCW + NCW],
                            start=(p == 0), stop=(p == NPACK - 1),
                        )
                        nc.tensor.matmul(
                            acc_w[:],
                            ones_sb[0:PPART, wcol:wcol + 128],
                            ee4[:, u * 2 * NCW + NCW:(u + 1) * 2 * NCW],
                            start=(p == 0), stop=(p == NPACK - 1),
                        )

            # ---- software-pipelined emission ----
            # an(j) at step j+1, exp(j) at j+2, H/ea/eh/acc(j) at j+3: each
            # cross-engine hop gets a full step of slack so the PE's in-order
            # queue (burst(t) then H(j)) never blocks on a late Exp.
            newton_done = [False] * len(PG)
            an_step: dict = {}
            exp_step: dict = {}
            n_an = 0
            n_exp = 0
            n_rest = 0
            t = 0
            while n_rest < NST:
                if (n_an < NST and n_an < t
                        and newton_done[group_of[n_an]]):
                    sweep2_an(n_an)
                    an_step[n_an] = t
                    n_an += 1
                elag = 1 if t < NST else 0
                if n_exp < n_an and an_step[n_exp] <= t - elag:
                    sweep2_exp(n_exp)
                    exp_step[n_exp] = t
                    n_exp += 1
                lag = 1 if t < NST else 0
                do_rest = (n_rest < n_exp and exp_step[n_rest] <= t - lag)
                if t >= 4:
                    pe_fill(6)
                if t < NST:
                    sweep1_mm(t)
                if do_rest:
                    sweep2_rest_a(n_rest)
                if t < NST:
                    sweep1_post(t)
                    g = group_of[t]
                    if t == PG[g][-1]:
                        newton(g)
                        newton_done[g] = True
                if do_rest:
                    sweep2_rest_b(n_rest)
                    n_rest += 1
                t += 1

            # ---- phase 2: sim = nz * nlc * rsqrt(wz), LSE over words ----
            # column-halved; newton chain halves on GPSIMD and DVE in
            # parallel so per-op semaphore latency overlaps.
            # split at the width-group boundary nearest NCW/2 so the LSE
            # reduces of the first groups only depend on the first half's
            # Exp (region-level dep tracking starts them early)
            bnds = [c0 for (_, _, _, c0) in groups][1:]
            H0 = min(bnds, key=lambda b: abs(b - NCW // 2)) if bnds else NCW // 2
            halves = [(0, H0), (H0, NCW - H0)]

            def pt(tag, dt=f32):
                return ph2.tile([BP, NCW], dt, tag=tag, name=f"ph2_{tag}")

            pt1 = pt("pt1", i32)
            py0 = pt("py0")
            rn = pt("rn")
            for (o, n) in halves:
                nc.vector.tensor_scalar(
                    pt1[:, o:o + n], wz_v[0:BP, o:o + n].bitcast(i32),
                    1, 1, op0=ALU.bitwise_or, op1=ALU.logical_shift_right)
            for (o, n) in halves:
                nc.vector.tensor_tensor(
                    py0[:, o:o + n].bitcast(i32),
                    magic[0:BP].broadcast_to([BP, n]),
                    pt1[:, o:o + n], op=ALU.subtract)
            for (o, n) in halves:
                nc.vector._custom_dve(
                    rsq, out=rn[:, o:o + n], in0=wz_v[0:BP, o:o + n],
                    in1=py0[:, o:o + n], s0=-0.5, s1=1.5,
                )
            # qq = rn * nlc; sim = qq * nz (PSUM -> DVE).  qq on DVE: at
            # phase-2 time DVE is drained while GPSIMD still holds the tail
            # an backlog (measured 4.8us queue wait when qq sat there).
            qq = pt("qq")
            for (o, n) in halves:
                nc.vector.tensor_mul(qq[:, o:o + n], rn[:, o:o + n],
                                     nlc_sb[:, o:o + n])
            sim = pt("sim")
            for (o, n) in halves:
                nc.vector.tensor_mul(sim[:, o:o + n], qq[:, o:o + n],
                                     nz_v[0:BP, o:o + n])
            ee = pt("ee")
            for (o, n) in halves:
                nc.scalar.activation(ee[:, o:o + n], sim[:, o:o + n],
                                     AF.Exp, scale=LAM_LSE)
            rowz = ph2.tile([BP, CPC], f32, tag="rowz")
            for (k0, ns, w, c0) in groups:
                nc.vector.tensor_reduce(
                    rowz[:, k0:k0 + ns],
                    ee[:, c0:c0 + ns * w].rearrange("p (c w) -> p c w", c=ns),
                    AX.X, ALU.add,
                )
            rowc = ph2.tile([BP, CPC], f32, tag="rowc")
            nc.vector.tensor_sub(rowc[:], rowz[:], pc_sb[:])
            nc.sync.dma_start(out_d[:], rowc[0:B, :])

    nc.compile()
    return nc


def prepare_inputs(im: np.ndarray, s: np.ndarray, s_l: np.ndarray):
    """Host-side marshalling: fp8 im packs + G/8, fp8 caption columns,
    onesbase window constant, 1/(cn*sqrt(8)) and pad counts."""
    import ml_dtypes

    bf16 = ml_dtypes.bfloat16
    fp8 = ml_dtypes.float8_e4m3
    im = np.ascontiguousarray(np.asarray(im, np.float32))
    s = np.ascontiguousarray(np.asarray(s, np.float32))
    s_l = np.asarray(s_l).astype(np.int64)

    widths, caps = choose_layout(s_l)
    NCW = sum(widths)
    U = 2 if NCW <= 256 else 1
    NST = NPACK // U

    # zero out padded words so A columns for padded (c, w) are exactly 0
    wmask = (np.arange(W)[None, :] < s_l[:, None])
    s_z = s * wmask[:, :, None].astype(np.float32)

    # im packs: [22, 128, 8*128], each 128-col chunk = 108 real + 20 zero
    imf = im.transpose(2, 0, 1).reshape(D, B * R)
    imf66 = np.zeros((D, BP * R), np.float32)
    imf66[:, : B * R] = imf
    im108 = (
        imf66.reshape(KCH, 128, NPACK, PPART)
        .transpose(2, 1, 0, 3)
        .reshape(NPACK, 128, KCH, PPART)
    )
    im_packed = np.zeros((NPACK, 128, KCH, SW), np.float32)
    im_packed[:, :, :, :PPART] = im108
    im_packed = im_packed.reshape(NPACK, 128, IMC)

    # Cholesky factors of the Gram matrices, block-diagonal per pack:
    # wz = E^T G E = ||L^T E||^2, so the device computes V = L^T E (same
    # matmul cost as G @ E) and squares it on ACT instead of an extra DVE
    # elementwise mul.  Shipped as L/sqrt(8) (fp8e4 range), wz' = wz/8 as
    # before.  Tiny jitter guards cholesky against near-singular G.
    G = np.matmul(im, im.transpose(0, 2, 1))
    L = np.linalg.cholesky(G + 1e-4 * np.eye(R)[None]) / np.sqrt(GSCALE)
    gbd = np.zeros((NPACK, 128, SW), np.float32)
    for jj in range(PACK):
        for p in range(NPACK):
            b = PACK * p + jj
            if b < B:
                gbd[p, R * jj: R * (jj + 1), R * jj: R * (jj + 1)] = L[b]

    pkb = np.zeros((NPACK, 128, PKC), np.float32)
    pkb[:, :, :IMC] = im_packed
    pkb[:, :, IMC:] = gbd
    pk8 = np.ascontiguousarray(
        pkb.reshape(NST, U, 128, PKC).transpose(0, 2, 1, 3)
        .reshape(NST, 128, U * PKC).astype(fp8)
    )

    # onesbase: [108, WOFF+128] bf16, ones at col WOFF + r//36
    onesb = np.zeros((128, WOFF + 128), np.float32)
    for r in range(PPART):
        onesb[r, WOFF + r // R] = 1.0
    onesb = np.ascontiguousarray(onesb.astype(bf16))

    cn = np.sqrt((s_z * s_z).sum(axis=2))
    nlc = np.where(cn > 0, 1.0 / np.maximum(cn, 1e-30), 0.0).astype(
        np.float32
    ) / np.sqrt(GSCALE)

    in_maps = []
    for c in range(NCORES):
        cc = caps[c]
        s_cols = np.concatenate(
            [s_z[cc[k], :widths[k], :] for k in range(CPC)], axis=0
        )                                                     # [ncw, 1024]
        sT = s_cols.T
        s8 = np.ascontiguousarray(
            sT.reshape(KCH, 128, NCW).transpose(1, 0, 2)
            .reshape(128, KCH * NCW).astype(fp8)
        )
        nlc_c = np.concatenate([nlc[cc[k], :widths[k]] for k in range(CPC)])
        padc_c = np.array(
            [widths[k] - s_l[cc[k]] for k in range(CPC)], np.float32
        )
        in_maps.append(
            {
                "pk8": pk8,
                "s8": s8,
                "onesb": onesb,
                "nlcv": np.ascontiguousarray(
                    np.broadcast_to(nlc_c.reshape(1, NCW), (BP, NCW)),
                    dtype=np.float32),
                "padcv": np.ascontiguousarray(
                    np.broadcast_to(padc_c.reshape(1, CPC), (BP, CPC)),
                    dtype=np.float32),
            }
        )
    return in_maps


def scores_from_results(res, s_l) -> np.ndarray:
    _, caps = choose_layout(s_l)
    scores = np.empty((B, B), np.float32)
    for c in range(NCORES):
        rowc = np.asarray(res[c]["scores8"], np.float32)      # [64, 8]
        sc = np.log(np.maximum(rowc, 1e-30)) / LAM_LSE
        for k in range(CPC):
            scores[:, caps[c, k]] = sc[:, k]
    return scores


def margin_loss(scores: np.ndarray) -> np.float32:
    scores = scores.astype(np.float32)
    diag = np.diag(scores).copy()
    cost_s = np.maximum(MARGIN + scores - diag[:, None], 0.0)
    cost_im = np.maximum(MARGIN + scores - diag[None, :], 0.0)
    np.fill_diagonal(cost_s, 0.0)
    np.fill_diagonal(cost_im, 0.0)
    return np.float32(cost_s.max(axis=1).sum() + cost_im.max(axis=0).sum())


def kernel(im: np.ndarray, s: np.ndarray, s_l: np.ndarray) -> np.ndarray:
    from concourse.bass_utils import run_bass_kernel_spmd

    widths, _ = choose_layout(s_l)
    if widths not in _PROGRAM_CACHE:
        _PROGRAM_CACHE[widths] = build_program(widths)
    nc = _PROGRAM_CACHE[widths]

    in_maps = prepare_inputs(im, s, s_l)
    res = run_bass_kernel_spmd(nc, in_maps, list(range(NCORES))).results
    return margin_loss(scores_from_results(res, s_l))


# revision 50
# speedup vs baseline: 1.0063x; 1.0063x over previous
"""Trainium2 Bass kernel for nn_ContrastiveLoss (ragged_sequence), v2.

Math (see reference): a cross-attention t2i score matrix scores[i, c] over
B=64 images x B=64 captions, then a max-violation margin loss.

Sharding: caption slots are sharded 8-per-core across 8 NeuronCores with a
four-width ragged slot layout (slot widths chosen per call from s_l and
compiled per layout); images are replicated.  Each core computes its
[64, 8] slot block of the score matrix; the host un-permutes slots and
runs the tiny margin reduction.

v2 design (~68us baseline -> target ~40us):
  * all A matmuls fp8e4 x fp8e4 (im, s, G/8 shipped fp8; measured loss
    error 1.7e-5); DMA traffic drops 8.5MB -> ~3.5MB per core.
  * four-width slot packing: NCW ~300 -> ~244 columns per core.
  * ea|eh concatenated per pack -> ONE accumulation matmul per pack
    (nz|wz side by side in a single PSUM bank); 20 matmuls per pair.
  * ones stationaries for the r-reduction are shifted 128-col windows of
    one tiny [108, 191] constant (nothing shipped per pack).
  * no raw-A Copy: ea reads A straight from PSUM (a_ps lives ~4 steps;
    PSUM budget 4+2+1 banks of 8).
  * engine split per pair: PE 20 matmuls; ACT Prelu+Exp; GPSIMD
    sq/newton-rsqrt/an; DVE word-norm reduces + ea + eh.
  * ~26 warmup matmuls on zeros during the input DMA keep HAM's clock
    gate busy so the first real burst runs at 2.4 GHz.
  * phase 2 (sim/LSE epilogue) is column-halved with the rsqrt newton on
    GPSIMD and the rest on DVE so semaphore latency overlaps.
  * Z-cancellation: sim = nz/(cn*sqrt(wz)) -- the softmax denominator
    cancels, so no Z accumulator, no reciprocals.
"""

import sys

if "/opt/trn_rl_repo" not in sys.path:
    sys.path.insert(0, "/opt/trn_rl_repo")

import numpy as np

B, R, W, D = 64, 36, 50, 1024
NCORES = 8
CPC = B // NCORES          # caption slots per core = 8
PACK = 3                   # images per pack
NPACK = 22                 # ceil(64 / 3) -> 66 rows incl 2 pad images
BP = NPACK * PACK          # 66
PPART = PACK * R           # 108 partitions per pack
KCH = D // 128             # 8 contraction chunks
SW = 128
IMC = KCH * SW             # 1024 im columns per pack (108 real + 20 pad)
PKC = IMC + SW             # 1152 pack columns (im | G)
WOFF = 63                  # onesbase window base column
GSCALE = 8.0               # G is shipped as G/8 (fp8e4 max ~240)
WARM_MM = 30               # PE warmup matmuls during input DMA

MARGIN = 0.2
LAM_SM = 9.0
LAM_LSE = 6.0
CLAMP_INT = 0x1E3CE508     # bits of f32 1e-20 (newton-seed zero guard)

_PROGRAM_CACHE: dict = {}

_RSQ_NAME = "ANT_RSQRT_NSTEP"


def _rsq_op():
    """Fused rsqrt Newton step as a custom DVE op:
    out = (sq(in1)*s0*in0 + s1)*in1  -- with in1 = magic-seed y0, in0 = x,
    (s0, s1) = (-4.5, 13.5) this is one Newton iteration of 9/sqrt(x).
    Registered into dve_ops.OPS at first use (documented extension point)."""
    import numpy as np
    import concourse.dve_ops as dve_ops
    for op in dve_ops.OPS:
        if op.name == _RSQ_NAME:
            return op
    from concourse.dve_spec import Spec, Src0, Src1, C0, C1, sq, lower
    from concourse.dve_spec import _has_src1
    from concourse.dve_uop import DveOpSpec

    # (sq(y0)*x)*c0 + c1)*y0 -- sq*x FIRST so x=0 (pad columns) zeroes the
    # huge seed before the c0 scale can overflow to inf
    spec = Spec(
        body=(sq(Src1) * Src0 * C0 + C1) * Src1,
        reference=lambda in0, in1, c0, c1, c2: (
            (in1.astype(np.float32) ** 2 * in0 * c0 + c1) * in1
        ),
    )
    row = dve_ops._CUSTOM_DVE_ROW_BASE + len(dve_ops.OPS)
    shas = {}
    for ver in ("v3", "v4"):
        s = DveOpSpec(name=_RSQ_NAME, opcode=row, uops=lower(spec, ver=ver),
                      rd1_en=_has_src1(spec))
        shas[ver] = s.sha(ver)
    op = dve_ops.DveOp(_RSQ_NAME, spec, subdim=False, uops_sha=shas)
    dve_ops.OPS.append(op)
    dve_ops._SUB_OPCODE_FOR_NAME[_RSQ_NAME] = row
    dve_ops.CUSTOM_DVE_SPECS[_RSQ_NAME] = spec
    return op


def choose_layout(s_l: np.ndarray):
    """Four-width caption slot packing.  Captions sorted by length are
    dealt round-robin: core c, slot k holds caption order[k*8+c].  Slot
    rank k needs width >= len(order[k*8+7]); ranks are grouped into <= 4
    contiguous groups sharing one (even) width, chosen to minimize NCW.
    Returns (widths, caps) with widths[k] = slot k's width."""
    s_l = np.asarray(s_l).astype(np.int64)
    order = np.argsort(s_l, kind="stable")
    caps = np.empty((NCORES, CPC), np.int64)
    for k in range(CPC):
        for c in range(NCORES):
            caps[c, k] = order[k * NCORES + c]
    need = [int(s_l[order[k * NCORES + NCORES - 1]]) for k in range(CPC)]

    best = None
    import itertools
    for nsplit in range(4):
        for cuts in itertools.combinations(range(1, CPC), nsplit):
            bounds = (0,) + cuts + (CPC,)
            widths = []
            for a, b in zip(bounds[:-1], bounds[1:]):
                w = max(need[a:b])
                widths += [w] * (b - a)
            ncw = sum(widths)
            if best is None or (ncw, nsplit) < best[0]:
                best = ((ncw, nsplit), tuple(widths))
    return best[1], caps


def slot_groups(widths):
    """Contiguous equal-width slot groups: list of (k0, nslots, w, col0)."""
    groups = []
    col = 0
    k = 0
    while k < CPC:
        k2 = k
        while k2 < CPC and widths[k2] == widths[k]:
            k2 += 1
        groups.append((k, k2 - k, widths[k], col))
        col += (k2 - k) * widths[k]
        k = k2
    return groups


def build_program(widths, debug: bool = False):
    import concourse.bacc as bacc
    import concourse.mybir as mybir
    import concourse.tile as tile

    f32 = mybir.dt.float32
    bf16 = mybir.dt.bfloat16
    fp8 = mybir.dt.float8e4
    i32 = mybir.dt.int32
    AF = mybir.ActivationFunctionType
    ALU = mybir.AluOpType
    AX = mybir.AxisListType

    NCW = sum(widths)
    groups = slot_groups(widths)
    U = 2 if NCW <= 256 else 1          # packs per step
    PBS = 256 if NCW <= 256 else 512    # per-pack PSUM stride
    NST = NPACK // U                    # pipeline steps (11 or 22)
    ACC1 = (2 * NCW <= 512)             # nz|wz share one PSUM bank

    # newton runs per step (singles): rn9(j) is ready at the end of step j,
    # so an(j) can run at step j+1 with a full step of slack
    PG = [[t] for t in range(NST)]
    group_of = {j: j for j in range(NST)}
    NMAX = U * CPC

    rsq = _rsq_op()

    nc = bacc.Bacc("TRN2", target_bir_lowering=False, debug=debug)

    pk_d = nc.dram_tensor("pk8", [NST, 128, U * PKC], fp8, kind="ExternalInput")
    s_d = nc.dram_tensor("s8", [128, KCH * NCW], fp8, kind="ExternalInput")
    ones_d = nc.dram_tensor("onesb", [128, WOFF + 128], bf16, kind="ExternalInput")
    nlc_d = nc.dram_tensor("nlcv", [BP, NCW], f32, kind="ExternalInput")
    pc_d = nc.dram_tensor("padcv", [BP, CPC], f32, kind="ExternalInput")
    out_d = nc.dram_tensor("scores8", [B, CPC], f32, kind="ExternalOutput")

    with tile.TileContext(nc) as tc:
        with (
            tc.tile_pool(name="const", bufs=1) as cpool,
            tc.tile_pool(name="pk", bufs=6) as pkpool,
            tc.tile_pool(name="ala", bufs=4) as alpool,
            tc.tile_pool(name="sqp", bufs=3) as sqpool,
            tc.tile_pool(name="anp", bufs=3) as anpool,
            tc.tile_pool(name="ep", bufs=4) as epool,
            tc.tile_pool(name="eaeh", bufs=3) as eapool,
            tc.tile_pool(name="nwt", bufs=2) as nwt,
            tc.tile_pool(name="ph2", bufs=2) as ph2,
            tc.tile_pool(name="psA", bufs=4, space="PSUM") as psA,
            tc.tile_pool(name="psH", bufs=2, space="PSUM") as psH,
            tc.tile_pool(name="psacc", bufs=1, space="PSUM") as psacc,
        ):
            s_sb = cpool.tile([128, KCH * NCW], fp8)
            ones_sb = cpool.tile([128, WOFF + 128], bf16)
            nlc_sb = cpool.tile([BP, NCW], f32)
            pc_sb = cpool.tile([BP, CPC], f32)

            magic = cpool.tile([PPART, 1], i32)
            nc.vector.memset(magic[:], 0x5F3759DF)
            wconst = cpool.tile([128, 128], bf16)
            nc.vector.memset(wconst[:], 0.0)

            s2_all = cpool.tile([PPART, NPACK * CPC], f32)
            rn9_all = cpool.tile([PPART, NPACK * CPC], f32)

            # persistent accumulators: [nz | wz] in one PSUM bank
            if ACC1:
                acc = psacc.tile([128, 2 * NCW], f32)
                nz_v = acc[:, 0:NCW]
                wz_v = acc[:, NCW:2 * NCW]
            else:
                acc_n = psacc.tile([128, NCW], f32)
                acc_w = psacc.tile([128, NCW], f32)
                nz_v, wz_v = acc_n[:], acc_w[:]
            wfill = psacc.tile([128, 128], f32, tag="wfill")

            # warmup matmuls on zeros: keep the PE's HAM activity window
            # busy while the first input DMAs land
            for _ in range(WARM_MM):
                nc.tensor.matmul(wfill[:], wconst[:], wconst[:],
                                 start=True, stop=True)

            def pe_fill(n):
                """Dependency-free matmuls into a scratch bank: absorb the
                PE idle gap while the burst waits on its PSUM tile so the
                HAM clock gate never sees an idle window (cold = half clock)."""
                for _ in range(n):
                    nc.tensor.matmul(wfill[:], wconst[:], wconst[:],
                                     start=True, stop=True)

            al_t: dict = {}
            pk_t: dict = {}
            e_t: dict = {}
            a_ps_t: dict = {}

            def pview(t_, n=NCW):
                return t_[0:PPART].rearrange(
                    "p (u x) -> p u x", u=U)[:, :, 0:n]

            def dma_pk(j):
                pk_sb = pkpool.tile([128, U * PKC], fp8, tag="pk")
                nc.sync.dma_start(pk_sb[:], pk_d[j])
                pk_t[j] = pk_sb

            def sweep1_mm(j):
                if j == 0:
                    nc.sync.dma_start(s_sb[:, :2 * NCW], s_d[:, :2 * NCW])
                    dma_pk(0)
                    dma_pk(1)
                    nc.sync.dma_start(ones_sb[:], ones_d[:])
                elif j + 1 < NST:
                    dma_pk(j + 1)
                if j == 1:
                    nc.sync.dma_start(nlc_sb[:], nlc_d[:])
                    nc.sync.dma_start(pc_sb[:], pc_d[:])
                pk_sb = pk_t[j]
                a_ps = psA.tile([128, U * PBS], f32)
                if j == 0:
                    for k in range(KCH):
                        if k == 2:
                            nc.sync.dma_start(s_sb[:, 2 * NCW:5 * NCW],
                                              s_d[:, 2 * NCW:5 * NCW])
                        if k == 5:
                            nc.sync.dma_start(s_sb[:, 5 * NCW:],
                                              s_d[:, 5 * NCW:])
                        for u in range(U):
                            nc.tensor.matmul(
                                a_ps[:, u * PBS:u * PBS + NCW],
                                pk_sb[:, u * PKC + k * SW:u * PKC + (k + 1) * SW],
                                s_sb[:, k * NCW:(k + 1) * NCW],
                                start=(k == 0), stop=(k == KCH - 1),
                            )
                else:
                    for u in range(U):
                        for k in range(KCH):
                            nc.tensor.matmul(
                                a_ps[:, u * PBS:u * PBS + NCW],
                                pk_sb[:, u * PKC + k * SW:u * PKC + (k + 1) * SW],
                                s_sb[:, k * NCW:(k + 1) * NCW],
                                start=(k == 0), stop=(k == KCH - 1),
                            )
                a_ps_t[j] = a_ps

            def sweep1_post(j):
                a_ps = a_ps_t[j]
                # al = leaky_relu(A, 0.1) (ACT, PSUM -> SBUF bf16)
                al = alpool.tile([PPART, U * NCW], bf16, tag="al")
                nc.scalar.activation(
                    al[:].rearrange("p (u x) -> p u x", u=U),
                    pview(a_ps), AF.Prelu, alpha=0.1,
                )
                al_t[j] = al
                # sq = al^2 (ACT Square), s2 = per-slot word sums (DVE)
                sq = sqpool.tile([PPART, U * NCW], bf16, tag="sq")
                nc.scalar.activation(sq[:], al[:], AF.Square)
                sqr = sq[:].rearrange("p (u x) -> p u x", u=U)
                s2r = s2_all[:, j * U * CPC:(j + 1) * U * CPC].rearrange(
                    "p (u c) -> p u c", u=U)
                for (k0, ns, w, c0) in groups:
                    nc.vector.tensor_reduce(
                        s2r[:, :, k0:k0 + ns],
                        sqr[:, :, c0:c0 + ns * w].rearrange(
                            "p u (c w) -> p u c w", c=ns),
                        AX.X, ALU.add,
                    )

            def newton(g):
                """rn9 = 9/sqrt(s2): fused int-clamp+shift (one
                tensor_scalar: int-max == float-max for positive floats),
                magic seed subtract, then ONE fused custom-DVE Newton step.
                The clamp keeps all-zero pad-image columns finite."""
                j = PG[g][0]
                lo, hi = j * U * CPC, (j + 1) * U * CPC
                n = hi - lo

                def tl(tag, dt=f32):
                    t_ = nwt.tile([PPART, NMAX], dt, tag=tag, name=f"nwt_{tag}")
                    return t_[:, :n]

                t1 = tl("t1", i32)
                nc.vector.tensor_scalar(
                    t1, s2_all[:, lo:hi].bitcast(i32), 1, 1,
                    op0=ALU.bitwise_or, op1=ALU.logical_shift_right
                )
                y0 = tl("y0")
                nc.gpsimd.tensor_tensor(
                    y0.bitcast(i32),
                    magic[:].broadcast_to([PPART, n]),
                    t1,
                    op=ALU.subtract,
                )
                nc.vector._custom_dve(
                    rsq, out=rn9_all[:, lo:hi], in0=s2_all[:, lo:hi],
                    in1=y0, s0=-4.5, s1=13.5,
                )

            an_t: dict = {}

            def sweep2_an(j):
                # an = al * rn9 broadcast over words (GPSIMD, per width-group).
                # For the tail pairs (no bursts left) the width-groups split
                # across GPSIMD and DVE: DVE runs disjoint-region writes
                # back-to-back, halving the an chain that gates exp -> H.
                al = al_t.pop(j)
                an = anpool.tile([PPART, U * NCW], f32, tag="an")
                anr = an[:].rearrange("p (u x) -> p u x", u=U)
                alr = al[:].rearrange("p (u x) -> p u x", u=U)
                rn = rn9_all[:, j * U * CPC:(j + 1) * U * CPC].rearrange(
                    "p (u c) -> p u c", u=U)
                for gi, (k0, ns, w, c0) in enumerate(groups):
                    eng = nc.gpsimd
                    eng.tensor_mul(
                        anr[:, :, c0:c0 + ns * w].rearrange(
                            "p u (c w) -> p u c w", c=ns),
                        alr[:, :, c0:c0 + ns * w].rearrange(
                            "p u (c w) -> p u c w", c=ns),
                        rn[:, :, k0:k0 + ns].broadcast_to([PPART, U, ns, w]),
                    )
                an_t[j] = an

            def sweep2_exp(j):
                an = an_t.pop(j)
                e = epool.tile([PPART, U * NCW], bf16, tag="e")
                nc.scalar.activation(e[:], an[:], AF.Exp)
                e_t[j] = e

            rest_st: dict = {}

            def sweep2_rest_a(j):
                """H matmuls + ea (DVE) -- emitted BEFORE sweep1_post(t) so
                the DVE queue runs ea/reduces/newton while the PE works
                toward H; eh (which needs H) is emitted after them."""
                pk_sb, e = pk_t.pop(j), e_t[j]
                a_ps = a_ps_t.pop(j)

                # H = (G/8) @ E per pack (fp8 stationary x bf16 moving)
                h_ps = psH.tile([128, U * PBS], f32, tag="h_ps")
                for u in range(U):
                    nc.tensor.matmul(
                        h_ps[:, u * PBS:u * PBS + NCW],
                        pk_sb[:PPART, u * PKC + IMC:u * PKC + IMC + SW],
                        e[:, u * NCW:(u + 1) * NCW],
                        start=True, stop=True,
                    )
                # ea | eh concatenated per pack: [108, U, 2, NCW]
                ee4 = eapool.tile([PPART, U * 2 * NCW], bf16, tag="ee4")
                er4 = ee4[:].rearrange("p (u k x) -> p u k x", u=U, k=2)
                nc.vector.tensor_mul(
                    er4[:, :, 0, :],
                    e[:].rearrange("p (u x) -> p u x", u=U),
                    pview(a_ps),
                )
                rest_st[j] = (e, h_ps, ee4)

            def sweep2_rest_b(j):
                e, h_ps, ee4 = rest_st.pop(j)
                e_t.pop(j)
                er4 = ee4[:].rearrange("p (u k x) -> p u k x", u=U, k=2)
                nc.vector.tensor_mul(
                    er4[:, :, 1, :],
                    e[:].rearrange("p (u x) -> p u x", u=U),
                    pview(h_ps),
                )
                # one accumulation matmul per pack: [nz | wz] += ones^T [ea|eh]
                for u in range(U):
                    p = U * j + u
                    wcol = WOFF - PACK * p
                    if ACC1:
                        nc.tensor.matmul(
                            acc[:],
                            ones_sb[0:PPART, wcol:wcol + 128],
                            ee4[:, u * 2 * NCW:(u + 1) * 2 * NCW],
                            start=(p == 0), stop=(p == NPACK - 1),
                        )
                    else:
                        nc.tensor.matmul(
                            acc_n[:],
                            ones_sb[0:PPART, wcol:wcol + 128],
                            ee4[:, u * 2 * NCW:u * 2 * NCW + NCW],
                            start=(p == 0), stop=(p == NPACK - 1),
                        )
                        nc.tensor.matmul(
                            acc_w[:],
                            ones_sb[0:PPART, wcol:wcol + 128],
                            ee4[:, u * 2 * NCW + NCW:(u + 1) * 2 * NCW],
                            start=(p == 0), stop=(p == NPACK - 1),
                        )

            # ---- software-pipelined emission ----
            # an(j) at step j+1, exp(j) at j+2, H/ea/eh/acc(j) at j+3: each
            # cross-engine hop gets a full step of slack so the PE's in-order
            # queue (burst(t) then H(j)) never blocks on a late Exp.
            newton_done = [False] * len(PG)
            an_step: dict = {}
            exp_step: dict = {}
            n_an = 0
            n_exp = 0
            n_rest = 0
            t = 0
            while n_rest < NST:
                if (n_an < NST and n_an < t
                        and newton_done[group_of[n_an]]):
                    sweep2_an(n_an)
                    an_step[n_an] = t
                    n_an += 1
                elag = 1 if t < NST else 0
                if n_exp < n_an and an_step[n_exp] <= t - elag:
                    sweep2_exp(n_exp)
                    exp_step[n_exp] = t
                    n_exp += 1
                lag = 1 if t < NST else 0
                do_rest = (n_rest < n_exp and exp_step[n_rest] <= t - lag)
                if t >= 4:
                    pe_fill(6)
                if t < NST:
                    sweep1_mm(t)
                # post+newton BEFORE ea/eh in the DVE queue: rn9(t) (which
                # gates an(t) next step, and with it the whole tail chain)
                # completes ~1.3us earlier, while ea(j) still lands a full
                # step ahead of its burst(j+4) PSUM-reuse deadline.
                if t < NST:
                    sweep1_post(t)
                    g = group_of[t]
                    if t == PG[g][-1]:
                        newton(g)
                        newton_done[g] = True
                if do_rest:
                    sweep2_rest_a(n_rest)
                    sweep2_rest_b(n_rest)
                    n_rest += 1
                t += 1

            # ---- phase 2: sim = nz * nlc * rsqrt(wz), LSE over words ----
            # column-halved; newton chain halves on GPSIMD and DVE in
            # parallel so per-op semaphore latency overlaps.
            # split at the width-group boundary nearest NCW/2 so the LSE
            # reduces of the first groups only depend on the first half's
            # Exp (region-level dep tracking starts them early)
            bnds = [c0 for (_, _, _, c0) in groups][1:]
            H0 = min(bnds, key=lambda b: abs(b - NCW // 2)) if bnds else NCW // 2
            halves = [(0, H0), (H0, NCW - H0)]

            def pt(tag, dt=f32):
                return ph2.tile([BP, NCW], dt, tag=tag, name=f"ph2_{tag}")

            pt1 = pt("pt1", i32)
            py0 = pt("py0")
            rn = pt("rn")
            for (o, n) in halves:
                nc.vector.tensor_scalar(
                    pt1[:, o:o + n], wz_v[0:BP, o:o + n].bitcast(i32),
                    1, 1, op0=ALU.bitwise_or, op1=ALU.logical_shift_right)
            for (o, n) in halves:
                nc.vector.tensor_tensor(
                    py0[:, o:o + n].bitcast(i32),
                    magic[0:BP].broadcast_to([BP, n]),
                    pt1[:, o:o + n], op=ALU.subtract)
            for (o, n) in halves:
                nc.vector._custom_dve(
                    rsq, out=rn[:, o:o + n], in0=wz_v[0:BP, o:o + n],
                    in1=py0[:, o:o + n], s0=-0.5, s1=1.5,
                )
            # qq = rn * nlc; sim = qq * nz (PSUM -> DVE).  qq on DVE: at
            # phase-2 time DVE is drained while GPSIMD still holds the tail
            # an backlog (measured 4.8us queue wait when qq sat there).
            qq = pt("qq")
            for (o, n) in halves:
                nc.vector.tensor_mul(qq[:, o:o + n], rn[:, o:o + n],
                                     nlc_sb[:, o:o + n])
            sim = pt("sim")
            for (o, n) in halves:
                nc.vector.tensor_mul(sim[:, o:o + n], qq[:, o:o + n],
                                     nz_v[0:BP, o:o + n])
            ee = pt("ee")
            for (o, n) in halves:
                nc.scalar.activation(ee[:, o:o + n], sim[:, o:o + n],
                                     AF.Exp, scale=LAM_LSE)
            rowz = ph2.tile([BP, CPC], f32, tag="rowz")
            for (k0, ns, w, c0) in groups:
                nc.vector.tensor_reduce(
                    rowz[:, k0:k0 + ns],
                    ee[:, c0:c0 + ns * w].rearrange("p (c w) -> p c w", c=ns),
                    AX.X, ALU.add,
                )
            rowc = ph2.tile([BP, CPC], f32, tag="rowc")
            nc.vector.tensor_sub(rowc[:], rowz[:], pc_sb[:])
            nc.sync.dma_start(out_d[:], rowc[0:B, :])

    nc.compile()
    return nc


def prepare_inputs(im: np.ndarray, s: np.ndarray, s_l: np.ndarray):
    """Host-side marshalling: fp8 im packs + G/8, fp8 caption columns,
    onesbase window constant, 1/(cn*sqrt(8)) and pad counts."""
    import ml_dtypes

    bf16 = ml_dtypes.bfloat16
    fp8 = ml_dtypes.float8_e4m3
    im = np.ascontiguousarray(np.asarray(im, np.float32))
    s = np.ascontiguousarray(np.asarray(s, np.float32))
    s_l = np.asarray(s_l).astype(np.int64)

    widths, caps = choose_layout(s_l)
    NCW = sum(widths)
    U = 2 if NCW <= 256 else 1
    NST = NPACK // U

    # zero out padded words so A columns for padded (c, w) are exactly 0
    wmask = (np.arange(W)[None, :] < s_l[:, None])
    s_z = s * wmask[:, :, None].astype(np.float32)

    # im packs: [22, 128, 8*128], each 128-col chunk = 108 real + 20 zero
    imf = im.transpose(2, 0, 1).reshape(D, B * R)
    imf66 = np.zeros((D, BP * R), np.float32)
    imf66[:, : B * R] = imf
    im108 = (
        imf66.reshape(KCH, 128, NPACK, PPART)
        .transpose(2, 1, 0, 3)
        .reshape(NPACK, 128, KCH, PPART)
    )
    im_packed = np.zeros((NPACK, 128, KCH, SW), np.float32)
    im_packed[:, :, :, :PPART] = im108
    im_packed = im_packed.reshape(NPACK, 128, IMC)

    # Gram matrices / 8, block-diagonal per pack: [22, 108, 128-pad]
    G = np.matmul(im, im.transpose(0, 2, 1)) / GSCALE
    gbd = np.zeros((NPACK, 128, SW), np.float32)
    for jj in range(PACK):
        for p in range(NPACK):
            b = PACK * p + jj
            if b < B:
                gbd[p, R * jj: R * (jj + 1), R * jj: R * (jj + 1)] = G[b]

    pkb = np.zeros((NPACK, 128, PKC), np.float32)
    pkb[:, :, :IMC] = im_packed
    pkb[:, :, IMC:] = gbd
    pk8 = np.ascontiguousarray(
        pkb.reshape(NST, U, 128, PKC).transpose(0, 2, 1, 3)
        .reshape(NST, 128, U * PKC).astype(fp8)
    )

    # onesbase: [108, WOFF+128] bf16, ones at col WOFF + r//36
    onesb = np.zeros((128, WOFF + 128), np.float32)
    for r in range(PPART):
        onesb[r, WOFF + r // R] = 1.0
    onesb = np.ascontiguousarray(onesb.astype(bf16))

    cn = np.sqrt((s_z * s_z).sum(axis=2))
    nlc = np.where(cn > 0, 1.0 / np.maximum(cn, 1e-30), 0.0).astype(
        np.float32
    ) / np.sqrt(GSCALE)

    in_maps = []
    for c in range(NCORES):
        cc = caps[c]
        s_cols = np.concatenate(
            [s_z[cc[k], :widths[k], :] for k in range(CPC)], axis=0
        )                                                     # [ncw, 1024]
        sT = s_cols.T
        s8 = np.ascontiguousarray(
            sT.reshape(KCH, 128, NCW).transpose(1, 0, 2)
            .reshape(128, KCH * NCW).astype(fp8)
        )
        nlc_c = np.concatenate([nlc[cc[k], :widths[k]] for k in range(CPC)])
        padc_c = np.array(
            [widths[k] - s_l[cc[k]] for k in range(CPC)], np.float32
        )
        in_maps.append(
            {
                "pk8": pk8,
                "s8": s8,
                "onesb": onesb,
                "nlcv": np.ascontiguousarray(
                    np.broadcast_to(nlc_c.reshape(1, NCW), (BP, NCW)),
                    dtype=np.float32),
                "padcv": np.ascontiguousarray(
                    np.broadcast_to(padc_c.reshape(1, CPC), (BP, CPC)),
                    dtype=np.float32),
            }
        )
    return in_maps


def scores_from_results(res, s_l) -> np.ndarray:
    _, caps = choose_layout(s_l)
    scores = np.empty((B, B), np.float32)
    for c in range(NCORES):
        rowc = np.asarray(res[c]["scores8"], np.float32)      # [64, 8]
        sc = np.log(np.maximum(rowc, 1e-30)) / LAM_LSE
        for k in range(CPC):
            scores[:, caps[c, k]] = sc[:, k]
    return scores


def margin_loss(scores: np.ndarray) -> np.float32:
    scores = scores.astype(np.float32)
    diag = np.diag(scores).copy()
    cost_s = np.maximum(MARGIN + scores - diag[:, None], 0.0)
    cost_im = np.maximum(MARGIN + scores - diag[None, :], 0.0)
    np.fill_diagonal(cost_s, 0.0)
    np.fill_diagonal(cost_im, 0.0)
    return np.float32(cost_s.max(axis=1).sum() + cost_im.max(axis=0).sum())


def kernel(im: np.ndarray, s: np.ndarray, s_l: np.ndarray) -> np.ndarray:
    from concourse.bass_utils import run_bass_kernel_spmd

    widths, _ = choose_layout(s_l)
    if widths not in _PROGRAM_CACHE:
        _PROGRAM_CACHE[widths] = build_program(widths)
    nc = _PROGRAM_CACHE[widths]

    in_maps = prepare_inputs(im, s, s_l)
    res = run_bass_kernel_spmd(nc, in_maps, list(range(NCORES))).results
    return margin_loss(scores_from_results(res, s_l))


# revision 51
# speedup vs baseline: 1.0199x; 1.0135x over previous
"""Trainium2 Bass kernel for nn_ContrastiveLoss (ragged_sequence), v2.

Math (see reference): a cross-attention t2i score matrix scores[i, c] over
B=64 images x B=64 captions, then a max-violation margin loss.

Sharding: caption slots are sharded 8-per-core across 8 NeuronCores with a
four-width ragged slot layout (slot widths chosen per call from s_l and
compiled per layout); images are replicated.  Each core computes its
[64, 8] slot block of the score matrix; the host un-permutes slots and
runs the tiny margin reduction.

v2 design (~68us baseline -> target ~40us):
  * all A matmuls fp8e4 x fp8e4 (im, s, G/8 shipped fp8; measured loss
    error 1.7e-5); DMA traffic drops 8.5MB -> ~3.5MB per core.
  * four-width slot packing: NCW ~300 -> ~244 columns per core.
  * ea|eh concatenated per pack -> ONE accumulation matmul per pack
    (nz|wz side by side in a single PSUM bank); 20 matmuls per pair.
  * ones stationaries for the r-reduction are shifted 128-col windows of
    one tiny [108, 191] constant (nothing shipped per pack).
  * no raw-A Copy: ea reads A straight from PSUM (a_ps lives ~4 steps;
    PSUM budget 4+2+1 banks of 8).
  * engine split per pair: PE 20 matmuls; ACT Prelu+Exp; GPSIMD
    sq/newton-rsqrt/an; DVE word-norm reduces + ea + eh.
  * ~26 warmup matmuls on zeros during the input DMA keep HAM's clock
    gate busy so the first real burst runs at 2.4 GHz.
  * phase 2 (sim/LSE epilogue) is column-halved with the rsqrt newton on
    GPSIMD and the rest on DVE so semaphore latency overlaps.
  * Z-cancellation: sim = nz/(cn*sqrt(wz)) -- the softmax denominator
    cancels, so no Z accumulator, no reciprocals.
"""

import sys

if "/opt/trn_rl_repo" not in sys.path:
    sys.path.insert(0, "/opt/trn_rl_repo")

import numpy as np

B, R, W, D = 64, 36, 50, 1024
NCORES = 8
CPC = B // NCORES          # caption slots per core = 8
PACK = 3                   # images per pack
NPACK = 22                 # ceil(64 / 3) -> 66 rows incl 2 pad images
BP = NPACK * PACK          # 66
PPART = PACK * R           # 108 partitions per pack
KCH = D // 128             # 8 contraction chunks
SW = 128
IMC = KCH * SW             # 1024 im columns per pack (108 real + 20 pad)
PKC = IMC + SW             # 1152 pack columns (im | G)
WOFF = 63                  # onesbase window base column
GSCALE = 8.0               # G is shipped as G/8 (fp8e4 max ~240)
WARM_MM = 30               # PE warmup matmuls during input DMA

MARGIN = 0.2
LAM_SM = 9.0
LAM_LSE = 6.0
CLAMP_INT = 0x1E3CE508     # bits of f32 1e-20 (newton-seed zero guard)

_PROGRAM_CACHE: dict = {}

_RSQ_NAME = "ANT_RSQRT_NSTEP"


def _rsq_op():
    """Fused rsqrt Newton step as a custom DVE op:
    out = (sq(in1)*s0*in0 + s1)*in1  -- with in1 = magic-seed y0, in0 = x,
    (s0, s1) = (-4.5, 13.5) this is one Newton iteration of 9/sqrt(x).
    Registered into dve_ops.OPS at first use (documented extension point)."""
    import numpy as np
    import concourse.dve_ops as dve_ops
    for op in dve_ops.OPS:
        if op.name == _RSQ_NAME:
            return op
    from concourse.dve_spec import Spec, Src0, Src1, C0, C1, sq, lower
    from concourse.dve_spec import _has_src1
    from concourse.dve_uop import DveOpSpec

    # (sq(y0)*x)*c0 + c1)*y0 -- sq*x FIRST so x=0 (pad columns) zeroes the
    # huge seed before the c0 scale can overflow to inf
    spec = Spec(
        body=(sq(Src1) * Src0 * C0 + C1) * Src1,
        reference=lambda in0, in1, c0, c1, c2: (
            (in1.astype(np.float32) ** 2 * in0 * c0 + c1) * in1
        ),
    )
    row = dve_ops._CUSTOM_DVE_ROW_BASE + len(dve_ops.OPS)
    shas = {}
    for ver in ("v3", "v4"):
        s = DveOpSpec(name=_RSQ_NAME, opcode=row, uops=lower(spec, ver=ver),
                      rd1_en=_has_src1(spec))
        shas[ver] = s.sha(ver)
    op = dve_ops.DveOp(_RSQ_NAME, spec, subdim=False, uops_sha=shas)
    dve_ops.OPS.append(op)
    dve_ops._SUB_OPCODE_FOR_NAME[_RSQ_NAME] = row
    dve_ops.CUSTOM_DVE_SPECS[_RSQ_NAME] = spec
    return op


def choose_layout(s_l: np.ndarray):
    """Four-width caption slot packing.  Captions sorted by length are
    dealt round-robin: core c, slot k holds caption order[k*8+c].  Slot
    rank k needs width >= len(order[k*8+7]); ranks are grouped into <= 4
    contiguous groups sharing one (even) width, chosen to minimize NCW.
    Returns (widths, caps) with widths[k] = slot k's width."""
    s_l = np.asarray(s_l).astype(np.int64)
    order = np.argsort(s_l, kind="stable")
    caps = np.empty((NCORES, CPC), np.int64)
    for k in range(CPC):
        for c in range(NCORES):
            caps[c, k] = order[k * NCORES + c]
    need = [int(s_l[order[k * NCORES + NCORES - 1]]) for k in range(CPC)]

    best = None
    import itertools
    for nsplit in range(4):
        for cuts in itertools.combinations(range(1, CPC), nsplit):
            bounds = (0,) + cuts + (CPC,)
            widths = []
            for a, b in zip(bounds[:-1], bounds[1:]):
                w = max(need[a:b])
                widths += [w] * (b - a)
            ncw = sum(widths)
            if best is None or (ncw, nsplit) < best[0]:
                best = ((ncw, nsplit), tuple(widths))
    return best[1], caps


def slot_groups(widths):
    """Contiguous equal-width slot groups: list of (k0, nslots, w, col0)."""
    groups = []
    col = 0
    k = 0
    while k < CPC:
        k2 = k
        while k2 < CPC and widths[k2] == widths[k]:
            k2 += 1
        groups.append((k, k2 - k, widths[k], col))
        col += (k2 - k) * widths[k]
        k = k2
    return groups


def build_program(widths, debug: bool = False):
    import concourse.bacc as bacc
    import concourse.mybir as mybir
    import concourse.tile as tile

    f32 = mybir.dt.float32
    bf16 = mybir.dt.bfloat16
    fp8 = mybir.dt.float8e4
    i32 = mybir.dt.int32
    AF = mybir.ActivationFunctionType
    ALU = mybir.AluOpType
    AX = mybir.AxisListType

    NCW = sum(widths)
    groups = slot_groups(widths)
    U = 2 if NCW <= 256 else 1          # packs per step
    PBS = 256 if NCW <= 256 else 512    # per-pack PSUM stride
    NST = NPACK // U                    # pipeline steps (11 or 22)
    ACC1 = (2 * NCW <= 512)             # nz|wz share one PSUM bank

    # newton runs per step (singles): rn9(j) is ready at the end of step j,
    # so an(j) can run at step j+1 with a full step of slack
    PG = [[t] for t in range(NST)]
    group_of = {j: j for j in range(NST)}
    NMAX = U * CPC

    rsq = _rsq_op()

    nc = bacc.Bacc("TRN2", target_bir_lowering=False, debug=debug)

    pk_d = nc.dram_tensor("pk8", [NST, 128, U * PKC], fp8, kind="ExternalInput")
    s_d = nc.dram_tensor("s8", [128, KCH * NCW], fp8, kind="ExternalInput")
    ones_d = nc.dram_tensor("onesb", [128, WOFF + 128], bf16, kind="ExternalInput")
    nlc_d = nc.dram_tensor("nlcv", [BP, NCW], f32, kind="ExternalInput")
    pc_d = nc.dram_tensor("padcv", [BP, CPC], f32, kind="ExternalInput")
    out_d = nc.dram_tensor("scores8", [B, CPC], f32, kind="ExternalOutput")

    with tile.TileContext(nc) as tc:
        with (
            tc.tile_pool(name="const", bufs=1) as cpool,
            tc.tile_pool(name="pk", bufs=6) as pkpool,
            tc.tile_pool(name="ala", bufs=4) as alpool,
            tc.tile_pool(name="sqp", bufs=3) as sqpool,
            tc.tile_pool(name="anp", bufs=3) as anpool,
            tc.tile_pool(name="ep", bufs=4) as epool,
            tc.tile_pool(name="eaeh", bufs=3) as eapool,
            tc.tile_pool(name="nwt", bufs=2) as nwt,
            tc.tile_pool(name="ph2", bufs=2) as ph2,
            tc.tile_pool(name="psA", bufs=4, space="PSUM") as psA,
            tc.tile_pool(name="psH", bufs=2, space="PSUM") as psH,
            tc.tile_pool(name="psacc", bufs=1, space="PSUM") as psacc,
        ):
            s_sb = cpool.tile([128, KCH * NCW], fp8)
            ones_sb = cpool.tile([128, WOFF + 128], bf16)
            nlc_sb = cpool.tile([BP, NCW], f32)
            pc_sb = cpool.tile([BP, CPC], f32)

            magic = cpool.tile([PPART, 1], i32)
            nc.vector.memset(magic[:], 0x5F3759DF)
            wconst = cpool.tile([128, 128], bf16)
            nc.vector.memset(wconst[:], 0.0)

            s2_all = cpool.tile([PPART, NPACK * CPC], f32)
            rn9_all = cpool.tile([PPART, NPACK * CPC], f32)

            # persistent accumulators: [nz | wz] in one PSUM bank
            if ACC1:
                acc = psacc.tile([128, 2 * NCW], f32)
                nz_v = acc[:, 0:NCW]
                wz_v = acc[:, NCW:2 * NCW]
            else:
                acc_n = psacc.tile([128, NCW], f32)
                acc_w = psacc.tile([128, NCW], f32)
                nz_v, wz_v = acc_n[:], acc_w[:]
            wfill = psacc.tile([128, 128], f32, tag="wfill")

            # warmup matmuls on zeros: keep the PE's HAM activity window
            # busy while the first input DMAs land
            for _ in range(WARM_MM):
                nc.tensor.matmul(wfill[:], wconst[:], wconst[:],
                                 start=True, stop=True)

            def pe_fill(n):
                """Dependency-free matmuls into a scratch bank: absorb the
                PE idle gap while the burst waits on its PSUM tile so the
                HAM clock gate never sees an idle window (cold = half clock)."""
                for _ in range(n):
                    nc.tensor.matmul(wfill[:], wconst[:], wconst[:],
                                     start=True, stop=True)

            al_t: dict = {}
            pk_t: dict = {}
            e_t: dict = {}
            a_ps_t: dict = {}

            def pview(t_, n=NCW):
                return t_[0:PPART].rearrange(
                    "p (u x) -> p u x", u=U)[:, :, 0:n]

            def dma_pk(j):
                pk_sb = pkpool.tile([128, U * PKC], fp8, tag="pk")
                nc.sync.dma_start(pk_sb[:], pk_d[j])
                pk_t[j] = pk_sb

            def sweep1_mm(j):
                if j == 0:
                    nc.sync.dma_start(s_sb[:, :2 * NCW], s_d[:, :2 * NCW])
                    dma_pk(0)
                    dma_pk(1)
                    nc.sync.dma_start(ones_sb[:], ones_d[:])
                elif j + 1 < NST:
                    dma_pk(j + 1)
                if j == 1:
                    nc.sync.dma_start(nlc_sb[:], nlc_d[:])
                    nc.sync.dma_start(pc_sb[:], pc_d[:])
                pk_sb = pk_t[j]
                a_ps = psA.tile([128, U * PBS], f32)
                if j == 0:
                    for k in range(KCH):
                        if k == 2:
                            nc.sync.dma_start(s_sb[:, 2 * NCW:5 * NCW],
                                              s_d[:, 2 * NCW:5 * NCW])
                        if k == 5:
                            nc.sync.dma_start(s_sb[:, 5 * NCW:],
                                              s_d[:, 5 * NCW:])
                        for u in range(U):
                            nc.tensor.matmul(
                                a_ps[:, u * PBS:u * PBS + NCW],
                                pk_sb[:, u * PKC + k * SW:u * PKC + (k + 1) * SW],
                                s_sb[:, k * NCW:(k + 1) * NCW],
                                start=(k == 0), stop=(k == KCH - 1),
                            )
                else:
                    for u in range(U):
                        for k in range(KCH):
                            nc.tensor.matmul(
                                a_ps[:, u * PBS:u * PBS + NCW],
                                pk_sb[:, u * PKC + k * SW:u * PKC + (k + 1) * SW],
                                s_sb[:, k * NCW:(k + 1) * NCW],
                                start=(k == 0), stop=(k == KCH - 1),
                            )
                a_ps_t[j] = a_ps

            def sweep1_post(j):
                a_ps = a_ps_t[j]
                # al = leaky_relu(A, 0.1) (ACT, PSUM -> SBUF bf16)
                al = alpool.tile([PPART, U * NCW], bf16, tag="al")
                nc.scalar.activation(
                    al[:].rearrange("p (u x) -> p u x", u=U),
                    pview(a_ps), AF.Prelu, alpha=0.1,
                )
                al_t[j] = al
                # sq = al^2 (ACT Square), s2 = per-slot word sums (DVE)
                sq = sqpool.tile([PPART, U * NCW], bf16, tag="sq")
                nc.scalar.activation(sq[:], al[:], AF.Square)
                sqr = sq[:].rearrange("p (u x) -> p u x", u=U)
                s2r = s2_all[:, j * U * CPC:(j + 1) * U * CPC].rearrange(
                    "p (u c) -> p u c", u=U)
                for (k0, ns, w, c0) in groups:
                    nc.vector.tensor_reduce(
                        s2r[:, :, k0:k0 + ns],
                        sqr[:, :, c0:c0 + ns * w].rearrange(
                            "p u (c w) -> p u c w", c=ns),
                        AX.X, ALU.add,
                    )

            def newton(g):
                """rn9 = 9/sqrt(s2): fused int-clamp+shift (one
                tensor_scalar: int-max == float-max for positive floats),
                magic seed subtract, then ONE fused custom-DVE Newton step.
                The clamp keeps all-zero pad-image columns finite."""
                j = PG[g][0]
                lo, hi = j * U * CPC, (j + 1) * U * CPC
                n = hi - lo

                def tl(tag, dt=f32):
                    t_ = nwt.tile([PPART, NMAX], dt, tag=tag, name=f"nwt_{tag}")
                    return t_[:, :n]

                t1 = tl("t1", i32)
                nc.vector.tensor_scalar(
                    t1, s2_all[:, lo:hi].bitcast(i32), 1, 1,
                    op0=ALU.bitwise_or, op1=ALU.logical_shift_right
                )
                y0 = tl("y0")
                nc.gpsimd.tensor_tensor(
                    y0.bitcast(i32),
                    magic[:].broadcast_to([PPART, n]),
                    t1,
                    op=ALU.subtract,
                )
                nc.vector._custom_dve(
                    rsq, out=rn9_all[:, lo:hi], in0=s2_all[:, lo:hi],
                    in1=y0, s0=-4.5, s1=13.5,
                )

            an_t: dict = {}

            def sweep2_an(j):
                # an = al * rn9 broadcast over words (GPSIMD, per width-group).
                # For the tail pairs (no bursts left) the width-groups split
                # across GPSIMD and DVE: DVE runs disjoint-region writes
                # back-to-back, halving the an chain that gates exp -> H.
                al = al_t.pop(j)
                an = anpool.tile([PPART, U * NCW], f32, tag="an")
                anr = an[:].rearrange("p (u x) -> p u x", u=U)
                alr = al[:].rearrange("p (u x) -> p u x", u=U)
                rn = rn9_all[:, j * U * CPC:(j + 1) * U * CPC].rearrange(
                    "p (u c) -> p u c", u=U)
                for gi, (k0, ns, w, c0) in enumerate(groups):
                    eng = nc.gpsimd
                    eng.tensor_mul(
                        anr[:, :, c0:c0 + ns * w].rearrange(
                            "p u (c w) -> p u c w", c=ns),
                        alr[:, :, c0:c0 + ns * w].rearrange(
                            "p u (c w) -> p u c w", c=ns),
                        rn[:, :, k0:k0 + ns].broadcast_to([PPART, U, ns, w]),
                    )
                an_t[j] = an

            def sweep2_exp(j):
                an = an_t.pop(j)
                e = epool.tile([PPART, U * NCW], bf16, tag="e")
                nc.scalar.activation(e[:], an[:], AF.Exp)
                e_t[j] = e

            rest_st: dict = {}

            def sweep2_rest_a(j):
                """H matmuls + ea (DVE) -- emitted BEFORE sweep1_post(t) so
                the DVE queue runs ea/reduces/newton while the PE works
                toward H; eh (which needs H) is emitted after them."""
                pk_sb, e = pk_t.pop(j), e_t[j]
                a_ps = a_ps_t.pop(j)

                # H = (G/8) @ E per pack (fp8 stationary x bf16 moving)
                h_ps = psH.tile([128, U * PBS], f32, tag="h_ps")
                for u in range(U):
                    nc.tensor.matmul(
                        h_ps[:, u * PBS:u * PBS + NCW],
                        pk_sb[:PPART, u * PKC + IMC:u * PKC + IMC + SW],
                        e[:, u * NCW:(u + 1) * NCW],
                        start=True, stop=True,
                    )
                # ea | eh concatenated per pack: [108, U, 2, NCW]
                ee4 = eapool.tile([PPART, U * 2 * NCW], bf16, tag="ee4")
                er4 = ee4[:].rearrange("p (u k x) -> p u k x", u=U, k=2)
                nc.vector.tensor_mul(
                    er4[:, :, 0, :],
                    e[:].rearrange("p (u x) -> p u x", u=U),
                    pview(a_ps),
                )
                rest_st[j] = (e, h_ps, ee4)

            def sweep2_rest_b(j):
                e, h_ps, ee4 = rest_st.pop(j)
                e_t.pop(j)
                er4 = ee4[:].rearrange("p (u k x) -> p u k x", u=U, k=2)
                nc.vector.tensor_mul(
                    er4[:, :, 1, :],
                    e[:].rearrange("p (u x) -> p u x", u=U),
                    pview(h_ps),
                )
                # one accumulation matmul per pack: [nz | wz] += ones^T [ea|eh]
                for u in range(U):
                    p = U * j + u
                    wcol = WOFF - PACK * p
                    if ACC1:
                        nc.tensor.matmul(
                            acc[:],
                            ones_sb[0:PPART, wcol:wcol + 128],
                            ee4[:, u * 2 * NCW:(u + 1) * 2 * NCW],
                            start=(p == 0), stop=(p == NPACK - 1),
                        )
                    else:
                        nc.tensor.matmul(
                            acc_n[:],
                            ones_sb[0:PPART, wcol:wcol + 128],
                            ee4[:, u * 2 * NCW:u * 2 * NCW + NCW],
                            start=(p == 0), stop=(p == NPACK - 1),
                        )
                        nc.tensor.matmul(
                            acc_w[:],
                            ones_sb[0:PPART, wcol:wcol + 128],
                            ee4[:, u * 2 * NCW + NCW:(u + 1) * 2 * NCW],
                            start=(p == 0), stop=(p == NPACK - 1),
                        )

            # ---- software-pipelined emission ----
            # an(j) at step j+1, exp(j) at j+2, H/ea/eh/acc(j) at j+3: each
            # cross-engine hop gets a full step of slack so the PE's in-order
            # queue (burst(t) then H(j)) never blocks on a late Exp.
            newton_done = [False] * len(PG)
            an_step: dict = {}
            exp_step: dict = {}
            n_an = 0
            n_exp = 0
            n_rest = 0
            t = 0
            while n_rest < NST:
                if (n_an < NST and n_an < t
                        and newton_done[group_of[n_an]]):
                    sweep2_an(n_an)
                    an_step[n_an] = t
                    n_an += 1
                elag = 1 if t < NST else 0
                if n_exp < n_an and an_step[n_exp] <= t - elag:
                    sweep2_exp(n_exp)
                    exp_step[n_exp] = t
                    n_exp += 1
                lag = 1 if t < NST else 0
                do_rest = (n_rest < n_exp and exp_step[n_rest] <= t - lag)
                if t >= 4:
                    pe_fill(6)
                if t < NST:
                    sweep1_mm(t)
                if do_rest:
                    sweep2_rest_a(n_rest)
                if t < NST:
                    sweep1_post(t)
                    g = group_of[t]
                    if t == PG[g][-1]:
                        newton(g)
                        newton_done[g] = True
                if do_rest:
                    sweep2_rest_b(n_rest)
                    n_rest += 1
                t += 1

            # ---- phase 2: sim = nz * nlc * rsqrt(wz), LSE over words ----
            # column-halved; newton chain halves on GPSIMD and DVE in
            # parallel so per-op semaphore latency overlaps.
            # split at the width-group boundary nearest NCW/2 so the LSE
            # reduces of the first groups only depend on the first half's
            # Exp (region-level dep tracking starts them early)
            bnds = [c0 for (_, _, _, c0) in groups][1:]
            H0 = min(bnds, key=lambda b: abs(b - NCW // 2)) if bnds else NCW // 2
            halves = [(0, H0), (H0, NCW - H0)]

            def pt(tag, dt=f32):
                return ph2.tile([BP, NCW], dt, tag=tag, name=f"ph2_{tag}")

            pt1 = pt("pt1", i32)
            py0 = pt("py0")
            rn = pt("rn")
            for (o, n) in halves:
                nc.vector.tensor_scalar(
                    pt1[:, o:o + n], wz_v[0:BP, o:o + n].bitcast(i32),
                    1, 1, op0=ALU.bitwise_or, op1=ALU.logical_shift_right)
            for (o, n) in halves:
                nc.vector.tensor_tensor(
                    py0[:, o:o + n].bitcast(i32),
                    magic[0:BP].broadcast_to([BP, n]),
                    pt1[:, o:o + n], op=ALU.subtract)
            for (o, n) in halves:
                nc.vector._custom_dve(
                    rsq, out=rn[:, o:o + n], in0=wz_v[0:BP, o:o + n],
                    in1=py0[:, o:o + n], s0=-0.5, s1=1.5,
                )
            # qq = rn * nlc; sim = qq * nz (PSUM -> DVE).  qq on DVE: at
            # phase-2 time DVE is drained while GPSIMD still holds the tail
            # an backlog (measured 4.8us queue wait when qq sat there).
            qq = pt("qq")
            for (o, n) in halves:
                nc.vector.tensor_mul(qq[:, o:o + n], rn[:, o:o + n],
                                     nlc_sb[:, o:o + n])
            sim = pt("sim")
            for (o, n) in halves:
                nc.vector.tensor_mul(sim[:, o:o + n], qq[:, o:o + n],
                                     nz_v[0:BP, o:o + n])
            ee = pt("ee")
            for (o, n) in halves:
                nc.scalar.activation(ee[:, o:o + n], sim[:, o:o + n],
                                     AF.Exp, scale=LAM_LSE)
            rowz = ph2.tile([BP, CPC], f32, tag="rowz")
            for (k0, ns, w, c0) in groups:
                nc.vector.tensor_reduce(
                    rowz[:, k0:k0 + ns],
                    ee[:, c0:c0 + ns * w].rearrange("p (c w) -> p c w", c=ns),
                    AX.X, ALU.add,
                )
            rowc = ph2.tile([BP, CPC], f32, tag="rowc")
            nc.vector.tensor_sub(rowc[:], rowz[:], pc_sb[:])
            nc.sync.dma_start(out_d[:], rowc[0:B, :])

    nc.compile()
    return nc


def prepare_inputs(im: np.ndarray, s: np.ndarray, s_l: np.ndarray):
    """Host-side marshalling: fp8 im packs + G/8, fp8 caption columns,
    onesbase window constant, 1/(cn*sqrt(8)) and pad counts."""
    import ml_dtypes

    bf16 = ml_dtypes.bfloat16
    fp8 = ml_dtypes.float8_e4m3
    im = np.ascontiguousarray(np.asarray(im, np.float32))
    s = np.ascontiguousarray(np.asarray(s, np.float32))
    s_l = np.asarray(s_l).astype(np.int64)

    widths, caps = choose_layout(s_l)
    NCW = sum(widths)
    U = 2 if NCW <= 256 else 1
    NST = NPACK // U

    # zero out padded words so A columns for padded (c, w) are exactly 0
    wmask = (np.arange(W)[None, :] < s_l[:, None])
    s_z = s * wmask[:, :, None].astype(np.float32)

    # im packs: [22, 128, 8*128], each 128-col chunk = 108 real + 20 zero
    imf = im.transpose(2, 0, 1).reshape(D, B * R)
    imf66 = np.zeros((D, BP * R), np.float32)
    imf66[:, : B * R] = imf
    im108 = (
        imf66.reshape(KCH, 128, NPACK, PPART)
        .transpose(2, 1, 0, 3)
        .reshape(NPACK, 128, KCH, PPART)
    )
    im_packed = np.zeros((NPACK, 128, KCH, SW), np.float32)
    im_packed[:, :, :, :PPART] = im108
    im_packed = im_packed.reshape(NPACK, 128, IMC)

    # Gram matrices / 8, block-diagonal per pack: [22, 108, 128-pad]
    G = np.matmul(im, im.transpose(0, 2, 1)) / GSCALE
    gbd = np.zeros((NPACK, 128, SW), np.float32)
    for jj in range(PACK):
        for p in range(NPACK):
            b = PACK * p + jj
            if b < B:
                gbd[p, R * jj: R * (jj + 1), R * jj: R * (jj + 1)] = G[b]

    pkb = np.zeros((NPACK, 128, PKC), np.float32)
    pkb[:, :, :IMC] = im_packed
    pkb[:, :, IMC:] = gbd
    pk8 = np.ascontiguousarray(
        pkb.reshape(NST, U, 128, PKC).transpose(0, 2, 1, 3)
        .reshape(NST, 128, U * PKC).astype(fp8)
    )

    # onesbase: [108, WOFF+128] bf16, ones at col WOFF + r//36
    onesb = np.zeros((128, WOFF + 128), np.float32)
    for r in range(PPART):
        onesb[r, WOFF + r // R] = 1.0
    onesb = np.ascontiguousarray(onesb.astype(bf16))

    cn = np.sqrt((s_z * s_z).sum(axis=2))
    nlc = np.where(cn > 0, 1.0 / np.maximum(cn, 1e-30), 0.0).astype(
        np.float32
    ) / np.sqrt(GSCALE)

    in_maps = []
    for c in range(NCORES):
        cc = caps[c]
        s_cols = np.concatenate(
            [s_z[cc[k], :widths[k], :] for k in range(CPC)], axis=0
        )                                                     # [ncw, 1024]
        sT = s_cols.T
        s8 = np.ascontiguousarray(
            sT.reshape(KCH, 128, NCW).transpose(1, 0, 2)
            .reshape(128, KCH * NCW).astype(fp8)
        )
        nlc_c = np.concatenate([nlc[cc[k], :widths[k]] for k in range(CPC)])
        padc_c = np.array(
            [widths[k] - s_l[cc[k]] for k in range(CPC)], np.float32
        )
        in_maps.append(
            {
                "pk8": pk8,
                "s8": s8,
                "onesb": onesb,
                "nlcv": np.ascontiguousarray(
                    np.broadcast_to(nlc_c.reshape(1, NCW), (BP, NCW)),
                    dtype=np.float32),
                "padcv": np.ascontiguousarray(
                    np.broadcast_to(padc_c.reshape(1, CPC), (BP, CPC)),
                    dtype=np.float32),
            }
        )
    return in_maps


def scores_from_results(res, s_l) -> np.ndarray:
    _, caps = choose_layout(s_l)
    scores = np.empty((B, B), np.float32)
    for c in range(NCORES):
        rowc = np.asarray(res[c]["scores8"], np.float32)      # [64, 8]
        sc = np.log(np.maximum(rowc, 1e-30)) / LAM_LSE
        for k in range(CPC):
            scores[:, caps[c, k]] = sc[:, k]
    return scores


def margin_loss(scores: np.ndarray) -> np.float32:
    scores = scores.astype(np.float32)
    diag = np.diag(scores).copy()
    cost_s = np.maximum(MARGIN + scores - diag[:, None], 0.0)
    cost_im = np.maximum(MARGIN + scores - diag[None, :], 0.0)
    np.fill_diagonal(cost_s, 0.0)
    np.fill_diagonal(cost_im, 0.0)
    return np.float32(cost_s.max(axis=1).sum() + cost_im.max(axis=0).sum())


def kernel(im: np.ndarray, s: np.ndarray, s_l: np.ndarray) -> np.ndarray:
    from concourse.bass_utils import run_bass_kernel_spmd

    widths, _ = choose_layout(s_l)
    if widths not in _PROGRAM_CACHE:
        _PROGRAM_CACHE[widths] = build_program(widths)
    nc = _PROGRAM_CACHE[widths]

    in_maps = prepare_inputs(im, s, s_l)
    res = run_bass_kernel_spmd(nc, in_maps, list(range(NCORES))).results
    return margin_loss(scores_from_results(res, s_l))


# revision 53
# speedup vs baseline: 1.0324x; 1.0122x over previous
"""Trainium2 Bass kernel for nn_ContrastiveLoss (ragged_sequence), v2.

Math (see reference): a cross-attention t2i score matrix scores[i, c] over
B=64 images x B=64 captions, then a max-violation margin loss.

Sharding: caption slots are sharded 8-per-core across 8 NeuronCores with a
four-width ragged slot layout (slot widths chosen per call from s_l and
compiled per layout); images are replicated.  Each core computes its
[64, 8] slot block of the score matrix; the host un-permutes slots and
runs the tiny margin reduction.

v2 design (~68us baseline -> target ~40us):
  * all A matmuls fp8e4 x fp8e4 (im, s, G/8 shipped fp8; measured loss
    error 1.7e-5); DMA traffic drops 8.5MB -> ~3.5MB per core.
  * four-width slot packing: NCW ~300 -> ~244 columns per core.
  * ea|eh concatenated per pack -> ONE accumulation matmul per pack
    (nz|wz side by side in a single PSUM bank); 20 matmuls per pair.
  * ones stationaries for the r-reduction are shifted 128-col windows of
    one tiny [108, 191] constant (nothing shipped per pack).
  * no raw-A Copy: ea reads A straight from PSUM (a_ps lives ~4 steps;
    PSUM budget 4+2+1 banks of 8).
  * engine split per pair: PE 20 matmuls; ACT Prelu+Exp; GPSIMD
    sq/newton-rsqrt/an; DVE word-norm reduces + ea + eh.
  * ~26 warmup matmuls on zeros during the input DMA keep HAM's clock
    gate busy so the first real burst runs at 2.4 GHz.
  * phase 2 (sim/LSE epilogue) is column-halved with the rsqrt newton on
    GPSIMD and the rest on DVE so semaphore latency overlaps.
  * Z-cancellation: sim = nz/(cn*sqrt(wz)) -- the softmax denominator
    cancels, so no Z accumulator, no reciprocals.
"""

import sys

if "/opt/trn_rl_repo" not in sys.path:
    sys.path.insert(0, "/opt/trn_rl_repo")

import numpy as np

B, R, W, D = 64, 36, 50, 1024
NCORES = 8
CPC = B // NCORES          # caption slots per core = 8
PACK = 3                   # images per pack
NPACK = 22                 # ceil(64 / 3) -> 66 rows incl 2 pad images
BP = NPACK * PACK          # 66
PPART = PACK * R           # 108 partitions per pack
KCH = D // 128             # 8 contraction chunks
SW = 128
IMC = KCH * SW             # 1024 im columns per pack (108 real + 20 pad)
PKC = IMC + SW             # 1152 pack columns (im | G)
WOFF = 63                  # onesbase window base column
GSCALE = 8.0               # G is shipped as G/8 (fp8e4 max ~240)
WARM_MM = 30               # PE warmup matmuls during input DMA

MARGIN = 0.2
LAM_SM = 9.0
LAM_LSE = 6.0
CLAMP_INT = 0x1E3CE508     # bits of f32 1e-20 (newton-seed zero guard)

_PROGRAM_CACHE: dict = {}

_RSQ_NAME = "ANT_RSQRT_NSTEP"


def _rsq_op():
    """Fused rsqrt Newton step as a custom DVE op:
    out = (sq(in1)*s0*in0 + s1)*in1  -- with in1 = magic-seed y0, in0 = x,
    (s0, s1) = (-4.5, 13.5) this is one Newton iteration of 9/sqrt(x).
    Registered into dve_ops.OPS at first use (documented extension point)."""
    import numpy as np
    import concourse.dve_ops as dve_ops
    for op in dve_ops.OPS:
        if op.name == _RSQ_NAME:
            return op
    from concourse.dve_spec import Spec, Src0, Src1, C0, C1, sq, lower
    from concourse.dve_spec import _has_src1
    from concourse.dve_uop import DveOpSpec

    # (sq(y0)*x)*c0 + c1)*y0 -- sq*x FIRST so x=0 (pad columns) zeroes the
    # huge seed before the c0 scale can overflow to inf
    spec = Spec(
        body=(sq(Src1) * Src0 * C0 + C1) * Src1,
        reference=lambda in0, in1, c0, c1, c2: (
            (in1.astype(np.float32) ** 2 * in0 * c0 + c1) * in1
        ),
    )
    row = dve_ops._CUSTOM_DVE_ROW_BASE + len(dve_ops.OPS)
    shas = {}
    for ver in ("v3", "v4"):
        s = DveOpSpec(name=_RSQ_NAME, opcode=row, uops=lower(spec, ver=ver),
                      rd1_en=_has_src1(spec))
        shas[ver] = s.sha(ver)
    op = dve_ops.DveOp(_RSQ_NAME, spec, subdim=False, uops_sha=shas)
    dve_ops.OPS.append(op)
    dve_ops._SUB_OPCODE_FOR_NAME[_RSQ_NAME] = row
    dve_ops.CUSTOM_DVE_SPECS[_RSQ_NAME] = spec
    return op


def choose_layout(s_l: np.ndarray):
    """Four-width caption slot packing.  Captions sorted by length are
    dealt round-robin: core c, slot k holds caption order[k*8+c].  Slot
    rank k needs width >= len(order[k*8+7]); ranks are grouped into <= 4
    contiguous groups sharing one (even) width, chosen to minimize NCW.
    Returns (widths, caps) with widths[k] = slot k's width."""
    s_l = np.asarray(s_l).astype(np.int64)
    order = np.argsort(s_l, kind="stable")
    caps = np.empty((NCORES, CPC), np.int64)
    for k in range(CPC):
        for c in range(NCORES):
            caps[c, k] = order[k * NCORES + c]
    need = [int(s_l[order[k * NCORES + NCORES - 1]]) for k in range(CPC)]

    best = None
    import itertools
    for nsplit in range(4):
        for cuts in itertools.combinations(range(1, CPC), nsplit):
            bounds = (0,) + cuts + (CPC,)
            widths = []
            for a, b in zip(bounds[:-1], bounds[1:]):
                w = max(need[a:b])
                widths += [w] * (b - a)
            ncw = sum(widths)
            if best is None or (ncw, nsplit) < best[0]:
                best = ((ncw, nsplit), tuple(widths))
    return best[1], caps


def slot_groups(widths):
    """Contiguous equal-width slot groups: list of (k0, nslots, w, col0)."""
    groups = []
    col = 0
    k = 0
    while k < CPC:
        k2 = k
        while k2 < CPC and widths[k2] == widths[k]:
            k2 += 1
        groups.append((k, k2 - k, widths[k], col))
        col += (k2 - k) * widths[k]
        k = k2
    return groups


def build_program(widths, debug: bool = False):
    import concourse.bacc as bacc
    import concourse.mybir as mybir
    import concourse.tile as tile

    f32 = mybir.dt.float32
    bf16 = mybir.dt.bfloat16
    fp8 = mybir.dt.float8e4
    i32 = mybir.dt.int32
    AF = mybir.ActivationFunctionType
    ALU = mybir.AluOpType
    AX = mybir.AxisListType

    NCW = sum(widths)
    groups = slot_groups(widths)
    U = 2 if NCW <= 256 else 1          # packs per step
    PBS = 256 if NCW <= 256 else 512    # per-pack PSUM stride
    NST = NPACK // U                    # pipeline steps (11 or 22)
    ACC1 = (2 * NCW <= 512)             # nz|wz share one PSUM bank

    # newton runs per step (singles): rn9(j) is ready at the end of step j,
    # so an(j) can run at step j+1 with a full step of slack
    PG = [[t] for t in range(NST)]
    group_of = {j: j for j in range(NST)}
    NMAX = U * CPC

    rsq = _rsq_op()

    nc = bacc.Bacc("TRN2", target_bir_lowering=False, debug=debug)

    pk_d = nc.dram_tensor("pk8", [NST, 128, U * PKC], fp8, kind="ExternalInput")
    s_d = nc.dram_tensor("s8", [128, KCH * NCW], fp8, kind="ExternalInput")
    ones_d = nc.dram_tensor("onesb", [128, WOFF + 128], bf16, kind="ExternalInput")
    nlc_d = nc.dram_tensor("nlcv", [BP, NCW], f32, kind="ExternalInput")
    out_d = nc.dram_tensor("scores8", [B, CPC], f32, kind="ExternalOutput")

    with tile.TileContext(nc) as tc:
        with (
            tc.tile_pool(name="const", bufs=1) as cpool,
            tc.tile_pool(name="pk", bufs=6) as pkpool,
            tc.tile_pool(name="ala", bufs=4) as alpool,
            tc.tile_pool(name="sqp", bufs=3) as sqpool,
            tc.tile_pool(name="anp", bufs=3) as anpool,
            tc.tile_pool(name="ep", bufs=4) as epool,
            tc.tile_pool(name="eaeh", bufs=3) as eapool,
            tc.tile_pool(name="nwt", bufs=2) as nwt,
            tc.tile_pool(name="ph2", bufs=2) as ph2,
            tc.tile_pool(name="psA", bufs=4, space="PSUM") as psA,
            tc.tile_pool(name="psH", bufs=2, space="PSUM") as psH,
            tc.tile_pool(name="psacc", bufs=1, space="PSUM") as psacc,
        ):
            s_sb = cpool.tile([128, KCH * NCW], fp8)
            ones_sb = cpool.tile([128, WOFF + 128], bf16)
            nlc_sb = cpool.tile([BP, NCW], f32)

            magic = cpool.tile([PPART, 1], i32)
            nc.vector.memset(magic[:], 0x5F3759DF)
            wconst = cpool.tile([128, 128], bf16)
            nc.vector.memset(wconst[:], 0.0)

            s2_all = cpool.tile([PPART, NPACK * CPC], f32)
            rn9_all = cpool.tile([PPART, NPACK * CPC], f32)

            # persistent accumulators: [nz | wz] in one PSUM bank
            if ACC1:
                acc = psacc.tile([128, 2 * NCW], f32)
                nz_v = acc[:, 0:NCW]
                wz_v = acc[:, NCW:2 * NCW]
            else:
                acc_n = psacc.tile([128, NCW], f32)
                acc_w = psacc.tile([128, NCW], f32)
                nz_v, wz_v = acc_n[:], acc_w[:]
            wfill = psacc.tile([128, 128], f32, tag="wfill")

            # warmup matmuls on zeros: keep the PE's HAM activity window
            # busy while the first input DMAs land
            for _ in range(WARM_MM):
                nc.tensor.matmul(wfill[:], wconst[:], wconst[:],
                                 start=True, stop=True)

            def pe_fill(n):
                """Dependency-free matmuls into a scratch bank: absorb the
                PE idle gap while the burst waits on its PSUM tile so the
                HAM clock gate never sees an idle window (cold = half clock)."""
                for _ in range(n):
                    nc.tensor.matmul(wfill[:], wconst[:], wconst[:],
                                     start=True, stop=True)

            al_t: dict = {}
            pk_t: dict = {}
            e_t: dict = {}
            a_ps_t: dict = {}

            def pview(t_, n=NCW):
                return t_[0:PPART].rearrange(
                    "p (u x) -> p u x", u=U)[:, :, 0:n]

            def dma_pk(j):
                pk_sb = pkpool.tile([128, U * PKC], fp8, tag="pk")
                nc.sync.dma_start(pk_sb[:], pk_d[j])
                pk_t[j] = pk_sb

            def sweep1_mm(j):
                if j == 0:
                    nc.sync.dma_start(s_sb[:, :2 * NCW], s_d[:, :2 * NCW])
                    dma_pk(0)
                    dma_pk(1)
                    nc.sync.dma_start(ones_sb[:], ones_d[:])
                elif j + 1 < NST:
                    dma_pk(j + 1)
                if j == 1:
                    nc.sync.dma_start(nlc_sb[:], nlc_d[:])
                pk_sb = pk_t[j]
                a_ps = psA.tile([128, U * PBS], f32)
                if j == 0:
                    for k in range(KCH):
                        if k == 2:
                            nc.sync.dma_start(s_sb[:, 2 * NCW:5 * NCW],
                                              s_d[:, 2 * NCW:5 * NCW])
                        if k == 5:
                            nc.sync.dma_start(s_sb[:, 5 * NCW:],
                                              s_d[:, 5 * NCW:])
                        for u in range(U):
                            nc.tensor.matmul(
                                a_ps[:, u * PBS:u * PBS + NCW],
                                pk_sb[:, u * PKC + k * SW:u * PKC + (k + 1) * SW],
                                s_sb[:, k * NCW:(k + 1) * NCW],
                                start=(k == 0), stop=(k == KCH - 1),
                            )
                else:
                    for u in range(U):
                        for k in range(KCH):
                            nc.tensor.matmul(
                                a_ps[:, u * PBS:u * PBS + NCW],
                                pk_sb[:, u * PKC + k * SW:u * PKC + (k + 1) * SW],
                                s_sb[:, k * NCW:(k + 1) * NCW],
                                start=(k == 0), stop=(k == KCH - 1),
                            )
                a_ps_t[j] = a_ps

            def sweep1_post(j):
                a_ps = a_ps_t[j]
                # al = leaky_relu(A, 0.1) (ACT, PSUM -> SBUF bf16)
                al = alpool.tile([PPART, U * NCW], bf16, tag="al")
                nc.scalar.activation(
                    al[:].rearrange("p (u x) -> p u x", u=U),
                    pview(a_ps), AF.Prelu, alpha=0.1,
                )
                al_t[j] = al
                # sq = al^2 (ACT Square), s2 = per-slot word sums (DVE)
                sq = sqpool.tile([PPART, U * NCW], bf16, tag="sq")
                nc.scalar.activation(sq[:], al[:], AF.Square)
                sqr = sq[:].rearrange("p (u x) -> p u x", u=U)
                s2r = s2_all[:, j * U * CPC:(j + 1) * U * CPC].rearrange(
                    "p (u c) -> p u c", u=U)
                for (k0, ns, w, c0) in groups:
                    nc.vector.tensor_reduce(
                        s2r[:, :, k0:k0 + ns],
                        sqr[:, :, c0:c0 + ns * w].rearrange(
                            "p u (c w) -> p u c w", c=ns),
                        AX.X, ALU.add,
                    )

            def newton(g):
                """rn9 = 9/sqrt(s2): fused int-clamp+shift (one
                tensor_scalar: int-max == float-max for positive floats),
                magic seed subtract, then ONE fused custom-DVE Newton step.
                The clamp keeps all-zero pad-image columns finite."""
                j = PG[g][0]
                lo, hi = j * U * CPC, (j + 1) * U * CPC
                n = hi - lo

                def tl(tag, dt=f32):
                    t_ = nwt.tile([PPART, NMAX], dt, tag=tag, name=f"nwt_{tag}")
                    return t_[:, :n]

                t1 = tl("t1", i32)
                nc.vector.tensor_scalar(
                    t1, s2_all[:, lo:hi].bitcast(i32), 1, 1,
                    op0=ALU.bitwise_or, op1=ALU.logical_shift_right
                )
                y0 = tl("y0")
                nc.gpsimd.tensor_tensor(
                    y0.bitcast(i32),
                    magic[:].broadcast_to([PPART, n]),
                    t1,
                    op=ALU.subtract,
                )
                nc.vector._custom_dve(
                    rsq, out=rn9_all[:, lo:hi], in0=s2_all[:, lo:hi],
                    in1=y0, s0=-4.5, s1=13.5,
                )

            an_t: dict = {}

            def sweep2_an(j):
                # an = al * rn9 broadcast over words (GPSIMD, per width-group).
                # For the tail pairs (no bursts left) the width-groups split
                # across GPSIMD and DVE: DVE runs disjoint-region writes
                # back-to-back, halving the an chain that gates exp -> H.
                al = al_t.pop(j)
                an = anpool.tile([PPART, U * NCW], f32, tag="an")
                anr = an[:].rearrange("p (u x) -> p u x", u=U)
                alr = al[:].rearrange("p (u x) -> p u x", u=U)
                rn = rn9_all[:, j * U * CPC:(j + 1) * U * CPC].rearrange(
                    "p (u c) -> p u c", u=U)
                for gi, (k0, ns, w, c0) in enumerate(groups):
                    eng = nc.gpsimd
                    eng.tensor_mul(
                        anr[:, :, c0:c0 + ns * w].rearrange(
                            "p u (c w) -> p u c w", c=ns),
                        alr[:, :, c0:c0 + ns * w].rearrange(
                            "p u (c w) -> p u c w", c=ns),
                        rn[:, :, k0:k0 + ns].broadcast_to([PPART, U, ns, w]),
                    )
                an_t[j] = an

            def sweep2_exp(j):
                an = an_t.pop(j)
                e = epool.tile([PPART, U * NCW], bf16, tag="e")
                nc.scalar.activation(e[:], an[:], AF.Exp)
                e_t[j] = e

            rest_st: dict = {}

            def sweep2_rest_a(j):
                """H matmuls + ea (DVE) -- emitted BEFORE sweep1_post(t) so
                the DVE queue runs ea/reduces/newton while the PE works
                toward H; eh (which needs H) is emitted after them."""
                pk_sb, e = pk_t.pop(j), e_t[j]
                a_ps = a_ps_t.pop(j)

                # H = (G/8) @ E per pack (fp8 stationary x bf16 moving)
                h_ps = psH.tile([128, U * PBS], f32, tag="h_ps")
                for u in range(U):
                    nc.tensor.matmul(
                        h_ps[:, u * PBS:u * PBS + NCW],
                        pk_sb[:PPART, u * PKC + IMC:u * PKC + IMC + SW],
                        e[:, u * NCW:(u + 1) * NCW],
                        start=True, stop=True,
                    )
                # ea | eh concatenated per pack: [108, U, 2, NCW]
                ee4 = eapool.tile([PPART, U * 2 * NCW], bf16, tag="ee4")
                er4 = ee4[:].rearrange("p (u k x) -> p u k x", u=U, k=2)
                nc.vector.tensor_mul(
                    er4[:, :, 0, :],
                    e[:].rearrange("p (u x) -> p u x", u=U),
                    pview(a_ps),
                )
                rest_st[j] = (e, h_ps, ee4)

            def sweep2_rest_b(j):
                e, h_ps, ee4 = rest_st.pop(j)
                e_t.pop(j)
                er4 = ee4[:].rearrange("p (u k x) -> p u k x", u=U, k=2)
                nc.vector.tensor_mul(
                    er4[:, :, 1, :],
                    e[:].rearrange("p (u x) -> p u x", u=U),
                    pview(h_ps),
                )
                # one accumulation matmul per pack: [nz | wz] += ones^T [ea|eh]
                for u in range(U):
                    p = U * j + u
                    wcol = WOFF - PACK * p
                    if ACC1:
                        nc.tensor.matmul(
                            acc[:],
                            ones_sb[0:PPART, wcol:wcol + 128],
                            ee4[:, u * 2 * NCW:(u + 1) * 2 * NCW],
                            start=(p == 0), stop=(p == NPACK - 1),
                        )
                    else:
                        nc.tensor.matmul(
                            acc_n[:],
                            ones_sb[0:PPART, wcol:wcol + 128],
                            ee4[:, u * 2 * NCW:u * 2 * NCW + NCW],
                            start=(p == 0), stop=(p == NPACK - 1),
                        )
                        nc.tensor.matmul(
                            acc_w[:],
                            ones_sb[0:PPART, wcol:wcol + 128],
                            ee4[:, u * 2 * NCW + NCW:(u + 1) * 2 * NCW],
                            start=(p == 0), stop=(p == NPACK - 1),
                        )

            # ---- software-pipelined emission ----
            # an(j) at step j+1, exp(j) at j+2, H/ea/eh/acc(j) at j+3: each
            # cross-engine hop gets a full step of slack so the PE's in-order
            # queue (burst(t) then H(j)) never blocks on a late Exp.
            newton_done = [False] * len(PG)
            an_step: dict = {}
            exp_step: dict = {}
            n_an = 0
            n_exp = 0
            n_rest = 0
            t = 0
            while n_rest < NST:
                if (n_an < NST and n_an < t
                        and newton_done[group_of[n_an]]):
                    sweep2_an(n_an)
                    an_step[n_an] = t
                    n_an += 1
                elag = 1 if t < NST else 0
                if n_exp < n_an and an_step[n_exp] <= t - elag:
                    sweep2_exp(n_exp)
                    exp_step[n_exp] = t
                    n_exp += 1
                lag = 1 if t < NST else 0
                do_rest = (n_rest < n_exp and exp_step[n_rest] <= t - lag)
                if t >= 4:
                    pe_fill(6)
                if t < NST:
                    sweep1_mm(t)
                if do_rest:
                    sweep2_rest_a(n_rest)
                if t < NST:
                    sweep1_post(t)
                    g = group_of[t]
                    if t == PG[g][-1]:
                        newton(g)
                        newton_done[g] = True
                if do_rest:
                    sweep2_rest_b(n_rest)
                    n_rest += 1
                t += 1

            # ---- phase 2: sim = nz * nlc * rsqrt(wz), LSE over words ----
            # column-halved; newton chain halves on GPSIMD and DVE in
            # parallel so per-op semaphore latency overlaps.
            # split at the width-group boundary nearest NCW/2 so the LSE
            # reduces of the first groups only depend on the first half's
            # Exp (region-level dep tracking starts them early)
            bnds = [c0 for (_, _, _, c0) in groups][1:]
            H0 = min(bnds, key=lambda b: abs(b - NCW // 2)) if bnds else NCW // 2
            halves = [(0, H0), (H0, NCW - H0)]

            def pt(tag, dt=f32):
                return ph2.tile([BP, NCW], dt, tag=tag, name=f"ph2_{tag}")

            pt1 = pt("pt1", i32)
            py0 = pt("py0")
            rn = pt("rn")
            for (o, n) in halves:
                nc.vector.tensor_scalar(
                    pt1[:, o:o + n], wz_v[0:BP, o:o + n].bitcast(i32),
                    1, 1, op0=ALU.bitwise_or, op1=ALU.logical_shift_right)
            for (o, n) in halves:
                nc.vector.tensor_tensor(
                    py0[:, o:o + n].bitcast(i32),
                    magic[0:BP].broadcast_to([BP, n]),
                    pt1[:, o:o + n], op=ALU.subtract)
            for (o, n) in halves:
                nc.vector._custom_dve(
                    rsq, out=rn[:, o:o + n], in0=wz_v[0:BP, o:o + n],
                    in1=py0[:, o:o + n], s0=-0.5, s1=1.5,
                )
            # qq = rn * nlc; sim = qq * nz (PSUM -> DVE).  qq on DVE: at
            # phase-2 time DVE is drained while GPSIMD still holds the tail
            # an backlog (measured 4.8us queue wait when qq sat there).
            qq = pt("qq")
            for (o, n) in halves:
                nc.vector.tensor_mul(qq[:, o:o + n], rn[:, o:o + n],
                                     nlc_sb[:, o:o + n])
            sim = pt("sim")
            for (o, n) in halves:
                nc.vector.tensor_mul(sim[:, o:o + n], qq[:, o:o + n],
                                     nz_v[0:BP, o:o + n])
            ee = pt("ee")
            for (o, n) in halves:
                nc.scalar.activation(ee[:, o:o + n], sim[:, o:o + n],
                                     AF.Exp, scale=LAM_LSE)
            rowz = ph2.tile([BP, CPC], f32, tag="rowz")
            for (k0, ns, w, c0) in groups:
                nc.vector.tensor_reduce(
                    rowz[:, k0:k0 + ns],
                    ee[:, c0:c0 + ns * w].rearrange("p (c w) -> p c w", c=ns),
                    AX.X, ALU.add,
                )
            # pad-word correction (rowz - padc) happens on the host, which
            # already post-processes with the log -- one less DVE op, sem
            # hop, input tensor, and DMA in the tail
            nc.sync.dma_start(out_d[:], rowz[0:B, :])

    nc.compile()
    return nc


def prepare_inputs(im: np.ndarray, s: np.ndarray, s_l: np.ndarray):
    """Host-side marshalling: fp8 im packs + G/8, fp8 caption columns,
    onesbase window constant, 1/(cn*sqrt(8)) and pad counts."""
    import ml_dtypes

    bf16 = ml_dtypes.bfloat16
    fp8 = ml_dtypes.float8_e4m3
    im = np.ascontiguousarray(np.asarray(im, np.float32))
    s = np.ascontiguousarray(np.asarray(s, np.float32))
    s_l = np.asarray(s_l).astype(np.int64)

    widths, caps = choose_layout(s_l)
    NCW = sum(widths)
    U = 2 if NCW <= 256 else 1
    NST = NPACK // U

    # zero out padded words so A columns for padded (c, w) are exactly 0
    wmask = (np.arange(W)[None, :] < s_l[:, None])
    s_z = s * wmask[:, :, None].astype(np.float32)

    # im packs: [22, 128, 8*128], each 128-col chunk = 108 real + 20 zero
    imf = im.transpose(2, 0, 1).reshape(D, B * R)
    imf66 = np.zeros((D, BP * R), np.float32)
    imf66[:, : B * R] = imf
    im108 = (
        imf66.reshape(KCH, 128, NPACK, PPART)
        .transpose(2, 1, 0, 3)
        .reshape(NPACK, 128, KCH, PPART)
    )
    im_packed = np.zeros((NPACK, 128, KCH, SW), np.float32)
    im_packed[:, :, :, :PPART] = im108
    im_packed = im_packed.reshape(NPACK, 128, IMC)

    # Gram matrices / 8, block-diagonal per pack: [22, 108, 128-pad]
    G = np.matmul(im, im.transpose(0, 2, 1)) / GSCALE
    gbd = np.zeros((NPACK, 128, SW), np.float32)
    for jj in range(PACK):
        for p in range(NPACK):
            b = PACK * p + jj
            if b < B:
                gbd[p, R * jj: R * (jj + 1), R * jj: R * (jj + 1)] = G[b]

    pkb = np.zeros((NPACK, 128, PKC), np.float32)
    pkb[:, :, :IMC] = im_packed
    pkb[:, :, IMC:] = gbd
    pk8 = np.ascontiguousarray(
        pkb.reshape(NST, U, 128, PKC).transpose(0, 2, 1, 3)
        .reshape(NST, 128, U * PKC).astype(fp8)
    )

    # onesbase: [108, WOFF+128] bf16, ones at col WOFF + r//36
    onesb = np.zeros((128, WOFF + 128), np.float32)
    for r in range(PPART):
        onesb[r, WOFF + r // R] = 1.0
    onesb = np.ascontiguousarray(onesb.astype(bf16))

    cn = np.sqrt((s_z * s_z).sum(axis=2))
    nlc = np.where(cn > 0, 1.0 / np.maximum(cn, 1e-30), 0.0).astype(
        np.float32
    ) / np.sqrt(GSCALE)

    in_maps = []
    for c in range(NCORES):
        cc = caps[c]
        s_cols = np.concatenate(
            [s_z[cc[k], :widths[k], :] for k in range(CPC)], axis=0
        )                                                     # [ncw, 1024]
        sT = s_cols.T
        s8 = np.ascontiguousarray(
            sT.reshape(KCH, 128, NCW).transpose(1, 0, 2)
            .reshape(128, KCH * NCW).astype(fp8)
        )
        nlc_c = np.concatenate([nlc[cc[k], :widths[k]] for k in range(CPC)])
        padc_c = np.array(
            [widths[k] - s_l[cc[k]] for k in range(CPC)], np.float32
        )
        in_maps.append(
            {
                "pk8": pk8,
                "s8": s8,
                "onesb": onesb,
                "nlcv": np.ascontiguousarray(
                    np.broadcast_to(nlc_c.reshape(1, NCW), (BP, NCW)),
                    dtype=np.float32),
            }
        )
    return in_maps


def scores_from_results(res, s_l) -> np.ndarray:
    s_l = np.asarray(s_l).astype(np.int64)
    widths, caps = choose_layout(s_l)
    scores = np.empty((B, B), np.float32)
    for c in range(NCORES):
        rowz = np.asarray(res[c]["scores8"], np.float32)      # [64, 8]
        padc = np.array([widths[k] - s_l[caps[c, k]] for k in range(CPC)],
                        np.float32)
        sc = np.log(np.maximum(rowz - padc[None, :], 1e-30)) / LAM_LSE
        for k in range(CPC):
            scores[:, caps[c, k]] = sc[:, k]
    return scores


def margin_loss(scores: np.ndarray) -> np.float32:
    scores = scores.astype(np.float32)
    diag = np.diag(scores).copy()
    cost_s = np.maximum(MARGIN + scores - diag[:, None], 0.0)
    cost_im = np.maximum(MARGIN + scores - diag[None, :], 0.0)
    np.fill_diagonal(cost_s, 0.0)
    np.fill_diagonal(cost_im, 0.0)
    return np.float32(cost_s.max(axis=1).sum() + cost_im.max(axis=0).sum())


def kernel(im: np.ndarray, s: np.ndarray, s_l: np.ndarray) -> np.ndarray:
    from concourse.bass_utils import run_bass_kernel_spmd

    widths, _ = choose_layout(s_l)
    if widths not in _PROGRAM_CACHE:
        _PROGRAM_CACHE[widths] = build_program(widths)
    nc = _PROGRAM_CACHE[widths]

    in_maps = prepare_inputs(im, s, s_l)
    res = run_bass_kernel_spmd(nc, in_maps, list(range(NCORES))).results
    return margin_loss(scores_from_results(res, s_l))


# revision 54
# speedup vs baseline: 1.0370x; 1.0045x over previous
"""Trainium2 Bass kernel for nn_ContrastiveLoss (ragged_sequence), v2.

Math (see reference): a cross-attention t2i score matrix scores[i, c] over
B=64 images x B=64 captions, then a max-violation margin loss.

Sharding: caption slots are sharded 8-per-core across 8 NeuronCores with a
four-width ragged slot layout (slot widths chosen per call from s_l and
compiled per layout); images are replicated.  Each core computes its
[64, 8] slot block of the score matrix; the host un-permutes slots and
runs the tiny margin reduction.

v2 design (~68us baseline -> target ~40us):
  * all A matmuls fp8e4 x fp8e4 (im, s, G/8 shipped fp8; measured loss
    error 1.7e-5); DMA traffic drops 8.5MB -> ~3.5MB per core.
  * four-width slot packing: NCW ~300 -> ~244 columns per core.
  * ea|eh concatenated per pack -> ONE accumulation matmul per pack
    (nz|wz side by side in a single PSUM bank); 20 matmuls per pair.
  * ones stationaries for the r-reduction are shifted 128-col windows of
    one tiny [108, 191] constant (nothing shipped per pack).
  * no raw-A Copy: ea reads A straight from PSUM (a_ps lives ~4 steps;
    PSUM budget 4+2+1 banks of 8).
  * engine split per pair: PE 20 matmuls; ACT Prelu+Exp; GPSIMD
    sq/newton-rsqrt/an; DVE word-norm reduces + ea + eh.
  * ~26 warmup matmuls on zeros during the input DMA keep HAM's clock
    gate busy so the first real burst runs at 2.4 GHz.
  * phase 2 (sim/LSE epilogue) is column-halved with the rsqrt newton on
    GPSIMD and the rest on DVE so semaphore latency overlaps.
  * Z-cancellation: sim = nz/(cn*sqrt(wz)) -- the softmax denominator
    cancels, so no Z accumulator, no reciprocals.
"""

import sys

if "/opt/trn_rl_repo" not in sys.path:
    sys.path.insert(0, "/opt/trn_rl_repo")

import numpy as np

B, R, W, D = 64, 36, 50, 1024
NCORES = 8
CPC = B // NCORES          # caption slots per core = 8
PACK = 3                   # images per pack
NPACK = 22                 # ceil(64 / 3) -> 66 rows incl 2 pad images
BP = NPACK * PACK          # 66
PPART = PACK * R           # 108 partitions per pack
KCH = D // 128             # 8 contraction chunks
SW = 128
IMC = KCH * SW             # 1024 im columns per pack (108 real + 20 pad)
PKC = IMC + SW             # 1152 pack columns (im | G)
WOFF = 63                  # onesbase window base column
GSCALE = 8.0               # G is shipped as G/8 (fp8e4 max ~240)
WARM_MM = 38               # PE warmup matmuls during input DMA (bridge the
                           # gap to the DMA-gated first burst so the HAM
                           # activity window never sees the PE idle)

MARGIN = 0.2
LAM_SM = 9.0
LAM_LSE = 6.0
CLAMP_INT = 0x1E3CE508     # bits of f32 1e-20 (newton-seed zero guard)

_PROGRAM_CACHE: dict = {}

_RSQ_NAME = "ANT_RSQRT_NSTEP"


def _rsq_op():
    """Fused rsqrt Newton step as a custom DVE op:
    out = (sq(in1)*s0*in0 + s1)*in1  -- with in1 = magic-seed y0, in0 = x,
    (s0, s1) = (-4.5, 13.5) this is one Newton iteration of 9/sqrt(x).
    Registered into dve_ops.OPS at first use (documented extension point)."""
    import numpy as np
    import concourse.dve_ops as dve_ops
    for op in dve_ops.OPS:
        if op.name == _RSQ_NAME:
            return op
    from concourse.dve_spec import Spec, Src0, Src1, C0, C1, sq, lower
    from concourse.dve_spec import _has_src1
    from concourse.dve_uop import DveOpSpec

    # (sq(y0)*x)*c0 + c1)*y0 -- sq*x FIRST so x=0 (pad columns) zeroes the
    # huge seed before the c0 scale can overflow to inf
    spec = Spec(
        body=(sq(Src1) * Src0 * C0 + C1) * Src1,
        reference=lambda in0, in1, c0, c1, c2: (
            (in1.astype(np.float32) ** 2 * in0 * c0 + c1) * in1
        ),
    )
    row = dve_ops._CUSTOM_DVE_ROW_BASE + len(dve_ops.OPS)
    shas = {}
    for ver in ("v3", "v4"):
        s = DveOpSpec(name=_RSQ_NAME, opcode=row, uops=lower(spec, ver=ver),
                      rd1_en=_has_src1(spec))
        shas[ver] = s.sha(ver)
    op = dve_ops.DveOp(_RSQ_NAME, spec, subdim=False, uops_sha=shas)
    dve_ops.OPS.append(op)
    dve_ops._SUB_OPCODE_FOR_NAME[_RSQ_NAME] = row
    dve_ops.CUSTOM_DVE_SPECS[_RSQ_NAME] = spec
    return op


def choose_layout(s_l: np.ndarray):
    """Four-width caption slot packing.  Captions sorted by length are
    dealt round-robin: core c, slot k holds caption order[k*8+c].  Slot
    rank k needs width >= len(order[k*8+7]); ranks are grouped into <= 4
    contiguous groups sharing one (even) width, chosen to minimize NCW.
    Returns (widths, caps) with widths[k] = slot k's width."""
    s_l = np.asarray(s_l).astype(np.int64)
    order = np.argsort(s_l, kind="stable")
    caps = np.empty((NCORES, CPC), np.int64)
    for k in range(CPC):
        for c in range(NCORES):
            caps[c, k] = order[k * NCORES + c]
    need = [int(s_l[order[k * NCORES + NCORES - 1]]) for k in range(CPC)]

    best = None
    import itertools
    for nsplit in range(4):
        for cuts in itertools.combinations(range(1, CPC), nsplit):
            bounds = (0,) + cuts + (CPC,)
            widths = []
            for a, b in zip(bounds[:-1], bounds[1:]):
                w = max(need[a:b])
                widths += [w] * (b - a)
            ncw = sum(widths)
            if best is None or (ncw, nsplit) < best[0]:
                best = ((ncw, nsplit), tuple(widths))
    return best[1], caps


def slot_groups(widths):
    """Contiguous equal-width slot groups: list of (k0, nslots, w, col0)."""
    groups = []
    col = 0
    k = 0
    while k < CPC:
        k2 = k
        while k2 < CPC and widths[k2] == widths[k]:
            k2 += 1
        groups.append((k, k2 - k, widths[k], col))
        col += (k2 - k) * widths[k]
        k = k2
    return groups


def build_program(widths, debug: bool = False):
    import concourse.bacc as bacc
    import concourse.mybir as mybir
    import concourse.tile as tile

    f32 = mybir.dt.float32
    bf16 = mybir.dt.bfloat16
    fp8 = mybir.dt.float8e4
    i32 = mybir.dt.int32
    AF = mybir.ActivationFunctionType
    ALU = mybir.AluOpType
    AX = mybir.AxisListType

    NCW = sum(widths)
    groups = slot_groups(widths)
    U = 2 if NCW <= 256 else 1          # packs per step
    PBS = 256 if NCW <= 256 else 512    # per-pack PSUM stride
    NST = NPACK // U                    # pipeline steps (11 or 22)
    ACC1 = (2 * NCW <= 512)             # nz|wz share one PSUM bank

    # newton runs per step (singles): rn9(j) is ready at the end of step j,
    # so an(j) can run at step j+1 with a full step of slack
    PG = [[t] for t in range(NST)]
    group_of = {j: j for j in range(NST)}
    NMAX = U * CPC

    rsq = _rsq_op()

    nc = bacc.Bacc("TRN2", target_bir_lowering=False, debug=debug)

    pk_d = nc.dram_tensor("pk8", [NST, 128, U * PKC], fp8, kind="ExternalInput")
    s_d = nc.dram_tensor("s8", [128, KCH * NCW], fp8, kind="ExternalInput")
    ones_d = nc.dram_tensor("onesb", [128, WOFF + 128], bf16, kind="ExternalInput")
    nlc_d = nc.dram_tensor("nlcv", [BP, NCW], f32, kind="ExternalInput")
    out_d = nc.dram_tensor("scores8", [B, CPC], f32, kind="ExternalOutput")

    with tile.TileContext(nc) as tc:
        with (
            tc.tile_pool(name="const", bufs=1) as cpool,
            tc.tile_pool(name="pk", bufs=6) as pkpool,
            tc.tile_pool(name="ala", bufs=4) as alpool,
            tc.tile_pool(name="sqp", bufs=3) as sqpool,
            tc.tile_pool(name="anp", bufs=3) as anpool,
            tc.tile_pool(name="ep", bufs=4) as epool,
            tc.tile_pool(name="eaeh", bufs=3) as eapool,
            tc.tile_pool(name="nwt", bufs=2) as nwt,
            tc.tile_pool(name="ph2", bufs=2) as ph2,
            tc.tile_pool(name="psA", bufs=4, space="PSUM") as psA,
            tc.tile_pool(name="psH", bufs=2, space="PSUM") as psH,
            tc.tile_pool(name="psacc", bufs=1, space="PSUM") as psacc,
        ):
            s_sb = cpool.tile([128, KCH * NCW], fp8)
            ones_sb = cpool.tile([128, WOFF + 128], bf16)
            nlc_sb = cpool.tile([BP, NCW], f32)

            magic = cpool.tile([PPART, 1], i32)
            nc.vector.memset(magic[:], 0x5F3759DF)
            wconst = cpool.tile([128, 128], bf16)
            nc.vector.memset(wconst[:], 0.0)

            s2_all = cpool.tile([PPART, NPACK * CPC], f32)
            rn9_all = cpool.tile([PPART, NPACK * CPC], f32)

            # persistent accumulators: [nz | wz] in one PSUM bank
            if ACC1:
                acc = psacc.tile([128, 2 * NCW], f32)
                nz_v = acc[:, 0:NCW]
                wz_v = acc[:, NCW:2 * NCW]
            else:
                acc_n = psacc.tile([128, NCW], f32)
                acc_w = psacc.tile([128, NCW], f32)
                nz_v, wz_v = acc_n[:], acc_w[:]
            wfill = psacc.tile([128, 128], f32, tag="wfill")

            # warmup matmuls on zeros: keep the PE's HAM activity window
            # busy while the first input DMAs land
            for _ in range(WARM_MM):
                nc.tensor.matmul(wfill[:], wconst[:], wconst[:],
                                 start=True, stop=True)

            def pe_fill(n):
                """Dependency-free matmuls into a scratch bank: absorb the
                PE idle gap while the burst waits on its PSUM tile so the
                HAM clock gate never sees an idle window (cold = half clock)."""
                for _ in range(n):
                    nc.tensor.matmul(wfill[:], wconst[:], wconst[:],
                                     start=True, stop=True)

            al_t: dict = {}
            pk_t: dict = {}
            e_t: dict = {}
            a_ps_t: dict = {}

            def pview(t_, n=NCW):
                return t_[0:PPART].rearrange(
                    "p (u x) -> p u x", u=U)[:, :, 0:n]

            def dma_pk(j):
                pk_sb = pkpool.tile([128, U * PKC], fp8, tag="pk")
                nc.sync.dma_start(pk_sb[:], pk_d[j])
                pk_t[j] = pk_sb

            def sweep1_mm(j):
                if j == 0:
                    nc.sync.dma_start(s_sb[:, :2 * NCW], s_d[:, :2 * NCW])
                    dma_pk(0)
                    dma_pk(1)
                    nc.sync.dma_start(ones_sb[:], ones_d[:])
                elif j + 1 < NST:
                    dma_pk(j + 1)
                if j == 1:
                    nc.sync.dma_start(nlc_sb[:], nlc_d[:])
                pk_sb = pk_t[j]
                a_ps = psA.tile([128, U * PBS], f32)
                if j == 0:
                    for k in range(KCH):
                        if k == 2:
                            nc.sync.dma_start(s_sb[:, 2 * NCW:5 * NCW],
                                              s_d[:, 2 * NCW:5 * NCW])
                        if k == 5:
                            nc.sync.dma_start(s_sb[:, 5 * NCW:],
                                              s_d[:, 5 * NCW:])
                        for u in range(U):
                            nc.tensor.matmul(
                                a_ps[:, u * PBS:u * PBS + NCW],
                                pk_sb[:, u * PKC + k * SW:u * PKC + (k + 1) * SW],
                                s_sb[:, k * NCW:(k + 1) * NCW],
                                start=(k == 0), stop=(k == KCH - 1),
                            )
                else:
                    for u in range(U):
                        for k in range(KCH):
                            nc.tensor.matmul(
                                a_ps[:, u * PBS:u * PBS + NCW],
                                pk_sb[:, u * PKC + k * SW:u * PKC + (k + 1) * SW],
                                s_sb[:, k * NCW:(k + 1) * NCW],
                                start=(k == 0), stop=(k == KCH - 1),
                            )
                a_ps_t[j] = a_ps

            def sweep1_post(j):
                a_ps = a_ps_t[j]
                # al = leaky_relu(A, 0.1) (ACT, PSUM -> SBUF bf16)
                al = alpool.tile([PPART, U * NCW], bf16, tag="al")
                nc.scalar.activation(
                    al[:].rearrange("p (u x) -> p u x", u=U),
                    pview(a_ps), AF.Prelu, alpha=0.1,
                )
                al_t[j] = al
                # sq = al^2 (ACT Square), s2 = per-slot word sums (DVE)
                sq = sqpool.tile([PPART, U * NCW], bf16, tag="sq")
                nc.scalar.activation(sq[:], al[:], AF.Square)
                sqr = sq[:].rearrange("p (u x) -> p u x", u=U)
                s2r = s2_all[:, j * U * CPC:(j + 1) * U * CPC].rearrange(
                    "p (u c) -> p u c", u=U)
                for (k0, ns, w, c0) in groups:
                    nc.vector.tensor_reduce(
                        s2r[:, :, k0:k0 + ns],
                        sqr[:, :, c0:c0 + ns * w].rearrange(
                            "p u (c w) -> p u c w", c=ns),
                        AX.X, ALU.add,
                    )

            def newton(g):
                """rn9 = 9/sqrt(s2): fused int-clamp+shift (one
                tensor_scalar: int-max == float-max for positive floats),
                magic seed subtract, then ONE fused custom-DVE Newton step.
                The clamp keeps all-zero pad-image columns finite."""
                j = PG[g][0]
                lo, hi = j * U * CPC, (j + 1) * U * CPC
                n = hi - lo

                def tl(tag, dt=f32):
                    t_ = nwt.tile([PPART, NMAX], dt, tag=tag, name=f"nwt_{tag}")
                    return t_[:, :n]

                t1 = tl("t1", i32)
                nc.vector.tensor_scalar(
                    t1, s2_all[:, lo:hi].bitcast(i32), 1, 1,
                    op0=ALU.bitwise_or, op1=ALU.logical_shift_right
                )
                y0 = tl("y0")
                nc.gpsimd.tensor_tensor(
                    y0.bitcast(i32),
                    magic[:].broadcast_to([PPART, n]),
                    t1,
                    op=ALU.subtract,
                )
                nc.vector._custom_dve(
                    rsq, out=rn9_all[:, lo:hi], in0=s2_all[:, lo:hi],
                    in1=y0, s0=-4.5, s1=13.5,
                )

            an_t: dict = {}

            def sweep2_an(j):
                # an = al * rn9 broadcast over words (GPSIMD, per width-group).
                # For the tail pairs (no bursts left) the width-groups split
                # across GPSIMD and DVE: DVE runs disjoint-region writes
                # back-to-back, halving the an chain that gates exp -> H.
                al = al_t.pop(j)
                an = anpool.tile([PPART, U * NCW], f32, tag="an")
                anr = an[:].rearrange("p (u x) -> p u x", u=U)
                alr = al[:].rearrange("p (u x) -> p u x", u=U)
                rn = rn9_all[:, j * U * CPC:(j + 1) * U * CPC].rearrange(
                    "p (u c) -> p u c", u=U)
                for gi, (k0, ns, w, c0) in enumerate(groups):
                    eng = nc.gpsimd
                    eng.tensor_mul(
                        anr[:, :, c0:c0 + ns * w].rearrange(
                            "p u (c w) -> p u c w", c=ns),
                        alr[:, :, c0:c0 + ns * w].rearrange(
                            "p u (c w) -> p u c w", c=ns),
                        rn[:, :, k0:k0 + ns].broadcast_to([PPART, U, ns, w]),
                    )
                an_t[j] = an

            def sweep2_exp(j):
                an = an_t.pop(j)
                e = epool.tile([PPART, U * NCW], bf16, tag="e")
                nc.scalar.activation(e[:], an[:], AF.Exp)
                e_t[j] = e

            rest_st: dict = {}

            def sweep2_rest_a(j):
                """H matmuls + ea (DVE) -- emitted BEFORE sweep1_post(t) so
                the DVE queue runs ea/reduces/newton while the PE works
                toward H; eh (which needs H) is emitted after them."""
                pk_sb, e = pk_t.pop(j), e_t[j]
                a_ps = a_ps_t.pop(j)

                # H = (G/8) @ E per pack (fp8 stationary x bf16 moving)
                h_ps = psH.tile([128, U * PBS], f32, tag="h_ps")
                for u in range(U):
                    nc.tensor.matmul(
                        h_ps[:, u * PBS:u * PBS + NCW],
                        pk_sb[:PPART, u * PKC + IMC:u * PKC + IMC + SW],
                        e[:, u * NCW:(u + 1) * NCW],
                        start=True, stop=True,
                    )
                # ea | eh concatenated per pack: [108, U, 2, NCW]
                ee4 = eapool.tile([PPART, U * 2 * NCW], bf16, tag="ee4")
                er4 = ee4[:].rearrange("p (u k x) -> p u k x", u=U, k=2)
                nc.vector.tensor_mul(
                    er4[:, :, 0, :],
                    e[:].rearrange("p (u x) -> p u x", u=U),
                    pview(a_ps),
                )
                rest_st[j] = (e, h_ps, ee4)

            def sweep2_rest_b(j):
                e, h_ps, ee4 = rest_st.pop(j)
                e_t.pop(j)
                er4 = ee4[:].rearrange("p (u k x) -> p u k x", u=U, k=2)
                nc.vector.tensor_mul(
                    er4[:, :, 1, :],
                    e[:].rearrange("p (u x) -> p u x", u=U),
                    pview(h_ps),
                )
                # one accumulation matmul per pack: [nz | wz] += ones^T [ea|eh]
                for u in range(U):
                    p = U * j + u
                    wcol = WOFF - PACK * p
                    if ACC1:
                        nc.tensor.matmul(
                            acc[:],
                            ones_sb[0:PPART, wcol:wcol + 128],
                            ee4[:, u * 2 * NCW:(u + 1) * 2 * NCW],
                            start=(p == 0), stop=(p == NPACK - 1),
                        )
                    else:
                        nc.tensor.matmul(
                            acc_n[:],
                            ones_sb[0:PPART, wcol:wcol + 128],
                            ee4[:, u * 2 * NCW:u * 2 * NCW + NCW],
                            start=(p == 0), stop=(p == NPACK - 1),
                        )
                        nc.tensor.matmul(
                            acc_w[:],
                            ones_sb[0:PPART, wcol:wcol + 128],
                            ee4[:, u * 2 * NCW + NCW:(u + 1) * 2 * NCW],
                            start=(p == 0), stop=(p == NPACK - 1),
                        )

            # ---- software-pipelined emission ----
            # an(j) at step j+1, exp(j) at j+2, H/ea/eh/acc(j) at j+3: each
            # cross-engine hop gets a full step of slack so the PE's in-order
            # queue (burst(t) then H(j)) never blocks on a late Exp.
            newton_done = [False] * len(PG)
            an_step: dict = {}
            exp_step: dict = {}
            n_an = 0
            n_exp = 0
            n_rest = 0
            t = 0
            while n_rest < NST:
                if (n_an < NST and n_an < t
                        and newton_done[group_of[n_an]]):
                    sweep2_an(n_an)
                    an_step[n_an] = t
                    n_an += 1
                elag = 1 if t < NST else 0
                if n_exp < n_an and an_step[n_exp] <= t - elag:
                    sweep2_exp(n_exp)
                    exp_step[n_exp] = t
                    n_exp += 1
                lag = 1 if t < NST else 0
                do_rest = (n_rest < n_exp and exp_step[n_rest] <= t - lag)
                if t >= 4:
                    pe_fill(6)
                if t < NST:
                    sweep1_mm(t)
                if do_rest:
                    sweep2_rest_a(n_rest)
                if t < NST:
                    sweep1_post(t)
                    g = group_of[t]
                    if t == PG[g][-1]:
                        newton(g)
                        newton_done[g] = True
                if do_rest:
                    sweep2_rest_b(n_rest)
                    n_rest += 1
                t += 1

            # ---- phase 2: sim = nz * nlc * rsqrt(wz), LSE over words ----
            # column-halved; newton chain halves on GPSIMD and DVE in
            # parallel so per-op semaphore latency overlaps.
            # split at the width-group boundary nearest NCW/2 so the LSE
            # reduces of the first groups only depend on the first half's
            # Exp (region-level dep tracking starts them early)
            bnds = [c0 for (_, _, _, c0) in groups][1:]
            H0 = min(bnds, key=lambda b: abs(b - NCW // 2)) if bnds else NCW // 2
            halves = [(0, H0), (H0, NCW - H0)]

            def pt(tag, dt=f32):
                return ph2.tile([BP, NCW], dt, tag=tag, name=f"ph2_{tag}")

            pt1 = pt("pt1", i32)
            py0 = pt("py0")
            rn = pt("rn")
            for (o, n) in halves:
                nc.vector.tensor_scalar(
                    pt1[:, o:o + n], wz_v[0:BP, o:o + n].bitcast(i32),
                    1, 1, op0=ALU.bitwise_or, op1=ALU.logical_shift_right)
            for (o, n) in halves:
                nc.vector.tensor_tensor(
                    py0[:, o:o + n].bitcast(i32),
                    magic[0:BP].broadcast_to([BP, n]),
                    pt1[:, o:o + n], op=ALU.subtract)
            for (o, n) in halves:
                nc.vector._custom_dve(
                    rsq, out=rn[:, o:o + n], in0=wz_v[0:BP, o:o + n],
                    in1=py0[:, o:o + n], s0=-0.5, s1=1.5,
                )
            # qq = rn * nlc; sim = qq * nz (PSUM -> DVE).  qq on DVE: at
            # phase-2 time DVE is drained while GPSIMD still holds the tail
            # an backlog (measured 4.8us queue wait when qq sat there).
            qq = pt("qq")
            for (o, n) in halves:
                nc.vector.tensor_mul(qq[:, o:o + n], rn[:, o:o + n],
                                     nlc_sb[:, o:o + n])
            sim = pt("sim")
            for (o, n) in halves:
                nc.vector.tensor_mul(sim[:, o:o + n], qq[:, o:o + n],
                                     nz_v[0:BP, o:o + n])
            ee = pt("ee")
            for (o, n) in halves:
                nc.scalar.activation(ee[:, o:o + n], sim[:, o:o + n],
                                     AF.Exp, scale=LAM_LSE)
            rowz = ph2.tile([BP, CPC], f32, tag="rowz")
            for (k0, ns, w, c0) in groups:
                nc.vector.tensor_reduce(
                    rowz[:, k0:k0 + ns],
                    ee[:, c0:c0 + ns * w].rearrange("p (c w) -> p c w", c=ns),
                    AX.X, ALU.add,
                )
            # pad-word correction (rowz - padc) happens on the host, which
            # already post-processes with the log -- one less DVE op, sem
            # hop, input tensor, and DMA in the tail
            nc.sync.dma_start(out_d[:], rowz[0:B, :])

    nc.compile()
    return nc


def prepare_inputs(im: np.ndarray, s: np.ndarray, s_l: np.ndarray):
    """Host-side marshalling: fp8 im packs + G/8, fp8 caption columns,
    onesbase window constant, 1/(cn*sqrt(8)) and pad counts."""
    import ml_dtypes

    bf16 = ml_dtypes.bfloat16
    fp8 = ml_dtypes.float8_e4m3
    im = np.ascontiguousarray(np.asarray(im, np.float32))
    s = np.ascontiguousarray(np.asarray(s, np.float32))
    s_l = np.asarray(s_l).astype(np.int64)

    widths, caps = choose_layout(s_l)
    NCW = sum(widths)
    U = 2 if NCW <= 256 else 1
    NST = NPACK // U

    # zero out padded words so A columns for padded (c, w) are exactly 0
    wmask = (np.arange(W)[None, :] < s_l[:, None])
    s_z = s * wmask[:, :, None].astype(np.float32)

    # im packs: [22, 128, 8*128], each 128-col chunk = 108 real + 20 zero
    imf = im.transpose(2, 0, 1).reshape(D, B * R)
    imf66 = np.zeros((D, BP * R), np.float32)
    imf66[:, : B * R] = imf
    im108 = (
        imf66.reshape(KCH, 128, NPACK, PPART)
        .transpose(2, 1, 0, 3)
        .reshape(NPACK, 128, KCH, PPART)
    )
    im_packed = np.zeros((NPACK, 128, KCH, SW), np.float32)
    im_packed[:, :, :, :PPART] = im108
    im_packed = im_packed.reshape(NPACK, 128, IMC)

    # Gram matrices / 8, block-diagonal per pack: [22, 108, 128-pad]
    G = np.matmul(im, im.transpose(0, 2, 1)) / GSCALE
    gbd = np.zeros((NPACK, 128, SW), np.float32)
    for jj in range(PACK):
        for p in range(NPACK):
            b = PACK * p + jj
            if b < B:
                gbd[p, R * jj: R * (jj + 1), R * jj: R * (jj + 1)] = G[b]

    pkb = np.zeros((NPACK, 128, PKC), np.float32)
    pkb[:, :, :IMC] = im_packed
    pkb[:, :, IMC:] = gbd
    pk8 = np.ascontiguousarray(
        pkb.reshape(NST, U, 128, PKC).transpose(0, 2, 1, 3)
        .reshape(NST, 128, U * PKC).astype(fp8)
    )

    # onesbase: [108, WOFF+128] bf16, ones at col WOFF + r//36
    onesb = np.zeros((128, WOFF + 128), np.float32)
    for r in range(PPART):
        onesb[r, WOFF + r // R] = 1.0
    onesb = np.ascontiguousarray(onesb.astype(bf16))

    cn = np.sqrt((s_z * s_z).sum(axis=2))
    nlc = np.where(cn > 0, 1.0 / np.maximum(cn, 1e-30), 0.0).astype(
        np.float32
    ) / np.sqrt(GSCALE)

    in_maps = []
    for c in range(NCORES):
        cc = caps[c]
        s_cols = np.concatenate(
            [s_z[cc[k], :widths[k], :] for k in range(CPC)], axis=0
        )                                                     # [ncw, 1024]
        sT = s_cols.T
        s8 = np.ascontiguousarray(
            sT.reshape(KCH, 128, NCW).transpose(1, 0, 2)
            .reshape(128, KCH * NCW).astype(fp8)
        )
        nlc_c = np.concatenate([nlc[cc[k], :widths[k]] for k in range(CPC)])
        padc_c = np.array(
            [widths[k] - s_l[cc[k]] for k in range(CPC)], np.float32
        )
        in_maps.append(
            {
                "pk8": pk8,
                "s8": s8,
                "onesb": onesb,
                "nlcv": np.ascontiguousarray(
                    np.broadcast_to(nlc_c.reshape(1, NCW), (BP, NCW)),
                    dtype=np.float32),
            }
        )
    return in_maps


def scores_from_results(res, s_l) -> np.ndarray:
    s_l = np.asarray(s_l).astype(np.int64)
    widths, caps = choose_layout(s_l)
    scores = np.empty((B, B), np.float32)
    for c in range(NCORES):
        rowz = np.asarray(res[c]["scores8"], np.float32)      # [64, 8]
        padc = np.array([widths[k] - s_l[caps[c, k]] for k in range(CPC)],
                        np.float32)
        sc = np.log(np.maximum(rowz - padc[None, :], 1e-30)) / LAM_LSE
        for k in range(CPC):
            scores[:, caps[c, k]] = sc[:, k]
    return scores


def margin_loss(scores: np.ndarray) -> np.float32:
    scores = scores.astype(np.float32)
    diag = np.diag(scores).copy()
    cost_s = np.maximum(MARGIN + scores - diag[:, None], 0.0)
    cost_im = np.maximum(MARGIN + scores - diag[None, :], 0.0)
    np.fill_diagonal(cost_s, 0.0)
    np.fill_diagonal(cost_im, 0.0)
    return np.float32(cost_s.max(axis=1).sum() + cost_im.max(axis=0).sum())


def kernel(im: np.ndarray, s: np.ndarray, s_l: np.ndarray) -> np.ndarray:
    from concourse.bass_utils import run_bass_kernel_spmd

    widths, _ = choose_layout(s_l)
    if widths not in _PROGRAM_CACHE:
        _PROGRAM_CACHE[widths] = build_program(widths)
    nc = _PROGRAM_CACHE[widths]

    in_maps = prepare_inputs(im, s, s_l)
    res = run_bass_kernel_spmd(nc, in_maps, list(range(NCORES))).results
    return margin_loss(scores_from_results(res, s_l))
